# revision 35
# baseline (speedup 1.0000x reference)
"""GNN message-passing kernel for Trainium2 (Bass/Tile), 8-core SPMD.

Fully sharded design (v2):
- Core r owns nodes [r*5120, (r+1)*5120); ALL dense compute is sharded 8x.
  BN batch stats are per-shard partial sums + a small AllReduce; the 960
  padding nodes (40000 -> 40960) are corrected analytically by tracking the
  (identical) pad-node column p through every transform.
- Neighbor aggregation uses matmul commutation:
      segment_sum((x@Wnb)[row], col) = segment_sum(x[row], col) @ Wnb
  so the gather table is x itself (node-major, AllGather'd once per layer);
  no per-layer h-table pass.
- Gather: dma_gather of 256B rows (int16 half-relative indices), scatter-add
  via PE one-hot matmuls (fp8 one-hots SBUF-resident, loaded once).
- Edge branch ea = segment_sum(edge_attr@Wedge + bedge, row) factors into
  rank-2 outer products of (s, c_out); its BN stats come in closed form from
  5 scalar moments of (s, c_out); s = segment_sum(edge_attr, row) is computed
  once on device by a row-sorted one-hot matmul pass and stays core-local.
- y1 = relu(bn_n(xn) + bn_a(aggr) + bn_e(ea)) is computed as a single PSUM
  accumulation: x@(Wn diag(sc_n)) + agx@(Wnb diag(sc_a)) + lhsT3-outer, with
  the scale-folded weights built on device via diag matmuls.
"""
import numpy as np
import ml_dtypes

F = 128
L = 3
EPS = 1e-5
NREAL = 40000
NE = 640000
NCORES = 8
NP_ = 40960
SHARD = NP_ // NCORES       # 5120
HALF = NP_ // 2             # 20480
NBLK = SHARD // F           # 40 blocks per core
CHUNK = 512
NCH = SHARD // CHUNK        # 10 dense chunks per core
NPAD = NP_ - NREAL          # 960

F16 = np.float16
FP8 = ml_dtypes.float8_e4m3


def _ceil(a, b):
    return -(-a // b)


def _wrap_idx16(vals):
    """int16 gather-index layout: value j at [j%16, j//16], tiled to 128 parts."""
    n = vals.shape[0]
    a = vals.reshape(n // 16, 16).T.astype(np.int16)   # [16, n/16]
    return np.tile(a, (8, 1))                          # [128, n/16]


def _prep(node_attr, edge_index, edge_attr):
    """Host-side index preprocessing -> per-core arrays + metadata."""
    row = edge_index[0].astype(np.int64)
    col = edge_index[1].astype(np.int64)
    ea = edge_attr[:, 0].astype(np.float32)

    shard = col // SHARD
    half = row // HALF
    blk = (col % SHARD) // F
    tloc = col % F

    # --- col pass (neighbor aggregation of raw x) ---
    cnt = np.zeros((NCORES, 2, NBLK), np.int64)
    np.add.at(cnt, (shard, half, blk), 1)
    sseg = _ceil(max(int(cnt.max()), 1), F) * F
    cpb = sseg // F                   # chunks per (half, block)
    nch_h = NBLK * cpb                # chunks per half
    nstream_h = NBLK * sseg           # slots per half
    nch = 2 * nch_h
    order = np.lexsort((blk, half, shard))
    so_shard, so_half, so_blk = shard[order], half[order], blk[order]
    grp = ((so_shard * 2 + so_half) * NBLK + so_blk)
    grp_start = np.zeros(NCORES * 2 * NBLK + 1, np.int64)
    np.add.at(grp_start, grp + 1, 1)
    grp_start = np.cumsum(grp_start)
    within = np.arange(NE) - grp_start[grp]
    slot = (so_half * NBLK + so_blk) * sseg + within   # slot in core stream

    gsrc = np.zeros((NCORES, 2 * nstream_h), np.int16)
    gsrc[so_shard, slot] = (row[order] - so_half * HALF).astype(np.int16)
    gidx = np.stack([_wrap_idx16(gsrc[r]) for r in range(NCORES)])  # [8,128,S/16]

    # one-hot flat for SBUF residency: [core, 128 part(slot%128), nch*F]
    ohc = np.zeros((NCORES, 128, nch * F), FP8)
    ohc[so_shard, slot % F, (slot // F) * F + tloc[order]] = 1.0

    # --- row pass (s = segment_sum(edge_attr, row)) ---
    rshard = row // SHARD
    rblk = (row % SHARD) // F
    rloc = row % F
    rcnt = np.zeros((NCORES, NBLK), np.int64)
    np.add.at(rcnt, (rshard, rblk), 1)
    rseg = _ceil(max(int(rcnt.max()), 1), F) * F
    cpb2 = rseg // F
    nch2 = NBLK * cpb2
    rorder = np.lexsort((rblk, rshard))
    ro_shard, ro_blk = rshard[rorder], rblk[rorder]
    rgrp = ro_shard * NBLK + ro_blk
    rgs = np.zeros(NCORES * NBLK + 1, np.int64)
    np.add.at(rgs, rgrp + 1, 1)
    rgs = np.cumsum(rgs)
    rwithin = np.arange(NE) - rgs[rgrp]
    rslot = ro_blk * rseg + rwithin

    eav = np.zeros((NCORES, NBLK * rseg), np.float32)
    eav[ro_shard, rslot] = ea[rorder]
    ohr = np.zeros((NCORES, nch2 // 8, 128, 8, F), FP8)
    ohr[ro_shard, (rslot // F) // 8, rslot % F, (rslot // F) % 8, rloc[rorder]] = 1.0
    eav_t = np.ascontiguousarray(
        eav.reshape(NCORES, nch2, F).transpose(0, 2, 1)).astype(F16)

    # degree counts (pure edge_index metadata), per-core shard slices
    c_out = np.bincount(row, minlength=NP_).astype(np.float32)
    c_in = np.bincount(col, minlength=NP_).astype(np.float32)
    cip = np.stack([c_in, c_out]).reshape(2, NCORES, SHARD).transpose(1, 0, 2)
    cip = np.ascontiguousarray(cip).astype(F16)   # [8, 2=(c_in,c_out), SHARD]

    naT = np.zeros((2, NP_), np.float32)
    naT[:, :NREAL] = node_attr.T
    naT = np.ascontiguousarray(
        naT.reshape(2, NCORES, SHARD).transpose(1, 0, 2)).astype(F16)  # [8,2,SHARD]

    return dict(sseg=sseg, cpb=cpb, nch=nch, nch_h=nch_h, nstream_h=nstream_h,
                rseg=rseg, cpb2=cpb2, nch2=nch2,
                gidx=gidx, ohc=ohc, ohr=ohr, eav_t=eav_t,
                cip=cip, naT=naT)


def _build(meta):
    """Build the Bass program."""
    import os
    DBG_NO_SCATTER = bool(int(os.environ.get("K_NO_SCATTER", "0")))
    DBG_NO_CC = bool(int(os.environ.get("K_NO_CC", "0")))
    import concourse.bass as bass
    import concourse.tile as tile
    from concourse import bacc, mybir

    sseg, cpb, nch_h = meta["sseg"], meta["cpb"], meta["nch_h"]
    nstream_h = meta["nstream_h"]
    nch = meta["nch"]
    cpb2, nch2 = meta["cpb2"], meta["nch2"]
    GCALL = 1024                      # 64 descriptors/engine = one packet
    NCALLS_H = nstream_h // GCALL     # = 5*cpb (45 for cpb=9)
    KPC = GCALL // F                  # chunks per gather call = 8
    dt = mybir.dt
    AX = mybir.AxisListType.X
    OP = mybir.AluOpType
    ACTF = mybir.ActivationFunctionType

    nc = bacc.Bacc("TRN2", target_bir_lowering=False, debug=False,
                   num_devices=NCORES, num_swdge_queues=4)

    def din(name, shape, d):
        return nc.dram_tensor(name, shape, d, kind="ExternalInput")

    naT = din("naT", [2, SHARD], dt.float16)
    cip = din("cip", [2, SHARD], dt.float16)
    gidx = din("gidx", [128, 2 * nstream_h // 16], dt.int16)
    ohc = din("ohc", [128, nch * F], dt.float8e4)
    ohr = din("ohr", [nch2 // 8, 128, 8, F], dt.float8e4)
    eav = din("eav", [128, nch2], dt.float16)
    W0 = din("W0", [2, F], dt.float16)
    Wn = [din(f"Wn{i}", [F, F], dt.float16) for i in range(L)]
    WnT = [din(f"WnT{i}", [F, F], dt.float16) for i in range(L)]
    Wb = [din(f"Wb{i}", [F, F], dt.float16) for i in range(L)]
    WbT = [din(f"WbT{i}", [F, F], dt.float16) for i in range(L)]
    W1 = [din(f"W1{i}", [F, F], dt.float16) for i in range(L)]
    W2 = [din(f"W2{i}", [F, F], dt.float16) for i in range(L)]
    WecA = [din(f"WecA{i}", [1, F], dt.float32) for i in range(L)]    # w_e row
    WecB = [din(f"WecB{i}", [1, F], dt.float32) for i in range(L)]    # b_e row
    WecC = [din(f"WecC{i}", [F, 2], dt.float32) for i in range(L)]    # cols
    BnbR = [din(f"BnbR{i}", [1, F], dt.float16) for i in range(L)]    # bnb row
    I16 = din("I16", [F, F], dt.float16)
    I32 = din("I32", [F, F], dt.float32)
    gcol = {}
    for nm in ("g0", "bt0"):
        gcol[nm] = din(nm, [F, 1], dt.float32)
    for i in range(L):
        for nm in ("gn", "btn", "ge", "bte", "gnb", "btnb",
                   "gm1", "btm1", "gm2", "btm2"):
            gcol[f"{nm}{i}"] = din(f"{nm}{i}", [F, 1], dt.float32)

    out = nc.dram_tensor("out", [F, SHARD], dt.float32, kind="ExternalOutput")

    with tile.TileContext(nc) as tc:
        import contextlib
        ctx = contextlib.ExitStack()
        with ctx:
            sb = ctx.enter_context(tc.tile_pool(name="sb", bufs=1))
            wpool = ctx.enter_context(tc.tile_pool(name="wp", bufs=1))
            tp = ctx.enter_context(tc.tile_pool(name="tp", bufs=2))
            gp = ctx.enter_context(tc.tile_pool(name="gp", bufs=2))
            ohrp = ctx.enter_context(tc.tile_pool(name="ohrp", bufs=2))
            stp = ctx.enter_context(tc.tile_pool(name="stp", bufs=1))
            afp = ctx.enter_context(tc.tile_pool(name="afp", bufs=4))
            ps_a = ctx.enter_context(tc.tile_pool(name="psa", bufs=2, space="PSUM"))
            ps_b = ctx.enter_context(tc.tile_pool(name="psb", bufs=2, space="PSUM"))
            ps_sc = ctx.enter_context(tc.tile_pool(name="pssc", bufs=2, space="PSUM"))
            ps_sm = ctx.enter_context(tc.tile_pool(name="pssm", bufs=1, space="PSUM"))
            ps_tp = ctx.enter_context(tc.tile_pool(name="pstp", bufs=1, space="PSUM"))
            dram = ctx.enter_context(tc.tile_pool(name="dram", bufs=1, space="DRAM"))

            # ---- persistent SBUF ----
            xbuf = sb.tile([F, SHARD], dt.float16)
            agx = sb.tile([F, SHARD], dt.float16)
            scv = sb.tile([3, SHARD], dt.float16)      # rows: c_in, s, c_out
            nc.sync.dma_start(scv[0:1, :], cip.ap()[0:1, :])
            nc.sync.dma_start(scv[2:3, :], cip.ap()[1:2, :])
            s_row = sb.tile([1, SHARD], dt.float16)
            cout_row = sb.tile([1, SHARD], dt.float16)
            nc.sync.dma_start(cout_row[:], cip.ap()[1:2, :])
            gidx_sb = sb.tile([128, 2 * nstream_h // 16], dt.int16)
            nc.sync.dma_start(gidx_sb[:], gidx.ap())
            eav_sb = sb.tile([128, nch2], dt.float16)
            nc.sync.dma_start(eav_sb[:], eav.ap())
            ohc_sb = sb.tile([128, nch * F], dt.float8e4)
            nc.sync.dma_start(ohc_sb[:], ohc.ap())
            trash = sb.tile([F, CHUNK], dt.float32)
            psmall = ps_sm.tile([F, F], dt.float32, tag="small")

            def wload(t_, tag):
                w = wpool.tile(list(t_.shape), t_.dtype, tag=tag)
                nc.sync.dma_start(w[:], t_.ap())
                return w

            W0_sb = wload(W0, "w0")
            I16_sb = wload(I16, "i16")
            I32_sb = wload(I32, "i32")
            Wn_sb = [wload(Wn[i], f"wn{i}") for i in range(L)]
            WnT_sb = [wload(WnT[i], f"wnt{i}") for i in range(L)]
            Wb_sb = [wload(Wb[i], f"wb{i}") for i in range(L)]
            WbT_sb = [wload(WbT[i], f"wbt{i}") for i in range(L)]
            W1_sb = [wload(W1[i], f"w1{i}") for i in range(L)]
            W2_sb = [wload(W2[i], f"w2{i}") for i in range(L)]
            WecA_sb = [wload(WecA[i], f"weca{i}") for i in range(L)]
            WecB_sb = [wload(WecB[i], f"wecb{i}") for i in range(L)]
            WecC_sb = [wload(WecC[i], f"wecc{i}") for i in range(L)]
            BnbR_sb = [wload(BnbR[i], f"bnbr{i}") for i in range(L)]
            gc_sb = {nm: wload(t_, f"p{nm}") for nm, t_ in gcol.items()}

            # ---- DRAM scratch ----
            ag_ins = [dram.tile([SHARD, F], dt.float16, tag=f"agi{i}",
                                name=f"agi{i}") for i in range(L)]
            agos = [dram.tile([NP_, F], dt.float16, addr_space="Shared",
                              tag=f"ago{i}", name=f"ago{i}") for i in range(L)]
            htabs = [dram.tile([HALF, F], dt.float16, tag=f"htab{h}",
                               name=f"htab{h}") for h in range(2)]
            ar_ins, ar_outs = [], []

            def make_ar(ncols, tag):
                i_ = dram.tile([F, ncols], dt.float32, tag=f"ari{tag}",
                               name=f"ari{tag}")
                o_ = dram.tile([F, ncols], dt.float32, addr_space="Shared",
                               tag=f"aro{tag}", name=f"aro{tag}")
                return i_, o_

            # ---------- helpers ----------
            def fire_ar(buf, ncols, tag):
                if DBG_NO_CC:
                    rb = stp.tile([F, ncols], dt.float32, tag=f"rb{tag}")
                    nc.vector.tensor_scalar_mul(rb[:], buf[:, :ncols],
                                                float(NCORES))
                    return rb
                ari, aro = make_ar(ncols, tag)
                nc.gpsimd.dma_start(ari[:], buf[:, :ncols])
                nc.gpsimd.collective_compute(
                    "AllReduce", OP.add, replica_groups=[list(range(NCORES))],
                    ins=[ari.opt()], outs=[aro.opt()])
                rb = stp.tile([F, ncols], dt.float32, tag=f"rb{tag}")
                nc.sync.dma_start(rb[:], aro[:])
                return rb

            def fin_mv(g, bt, mean, var):
                """(scale, shift) from mean/var columns [F,1]."""
                v2 = afp.tile([F, 1], dt.float32)
                nc.vector.tensor_scalar_add(v2[:], var[:], EPS)
                lnv = afp.tile([F, 1], dt.float32)
                nc.scalar.activation(lnv[:], v2[:], ACTF.Ln)
                isig = afp.tile([F, 1], dt.float32)
                nc.scalar.activation(isig[:], lnv[:], ACTF.Exp, scale=-0.5)
                scale = afp.tile([F, 1], dt.float32)
                nc.vector.tensor_mul(scale[:], g[:], isig[:])
                nscale = afp.tile([F, 1], dt.float32)
                nc.vector.tensor_scalar_mul(nscale[:], scale[:], -1.0)
                shift = afp.tile([F, 1], dt.float32)
                nc.vector.scalar_tensor_tensor(
                    out=shift[:], in0=mean[:], scalar=nscale[:], in1=bt[:],
                    op0=OP.mult, op1=OP.add)
                return scale, shift

            def fin_sums(g, bt, ssum, ssq, corr=None):
                """(scale, shift) from global sum/sumsq [F,1]; corr = pad col."""
                if corr is not None:
                    c2 = afp.tile([F, 1], dt.float32)
                    nc.scalar.activation(c2[:], corr[:], ACTF.Square)
                    nc.vector.scalar_tensor_tensor(
                        out=ssum[:], in0=corr[:], scalar=-float(NPAD), in1=ssum[:],
                        op0=OP.mult, op1=OP.add)
                    nc.vector.scalar_tensor_tensor(
                        out=ssq[:], in0=c2[:], scalar=-float(NPAD), in1=ssq[:],
                        op0=OP.mult, op1=OP.add)
                mean = afp.tile([F, 1], dt.float32)
                nc.vector.tensor_scalar_mul(mean[:], ssum[:], 1.0 / NREAL)
                m2t = afp.tile([F, 1], dt.float32)
                nc.scalar.activation(m2t[:], mean[:], ACTF.Square)
                var = afp.tile([F, 1], dt.float32)
                nc.vector.scalar_tensor_tensor(
                    out=var[:], in0=ssq[:], scalar=1.0 / NREAL, in1=m2t[:],
                    op0=OP.mult, op1=OP.subtract)
                sc, sh = fin_mv(g, bt, mean, var)
                return sc, sh, mean

            def stat_chunk(ps, c, ss, sq):
                nc.vector.tensor_reduce(ss[:, c:c + 1], ps[:, :], AX, OP.add)
                nc.scalar.activation(trash[:], ps[:, :], ACTF.Square,
                                     accum_out=sq[:, c:c + 1])

            def colred(sl):
                r = afp.tile([F, 1], dt.float32)
                nc.vector.tensor_reduce(r[:], sl[:], AX, OP.add)
                return r

            def col16(col):
                t = afp.tile([F, 1], dt.float16)
                nc.vector.tensor_copy(t[:], col[:])
                return t

            def pad_mm(w_sb, col_f16):
                """[F,1] = w.T @ col via PE; returns fp32 sbuf col."""
                nc.tensor.matmul(psmall[:, 0:1], lhsT=w_sb[:], rhs=col_f16[:],
                                 start=True, stop=True)
                o = afp.tile([F, 1], dt.float32)
                nc.vector.tensor_copy(o[:], psmall[:, 0:1])
                return o

            def scaled_w(wT_sb, sc):
                """W*diag(sc) as fp16 SBUF tile, via diag matmul."""
                dg = tp.tile([F, F], dt.float16, tag="diag")
                nc.vector.tensor_scalar_mul(dg[:], I32_sb[:], sc[:])
                nc.tensor.matmul(psmall[:], lhsT=wT_sb[:], rhs=dg[:],
                                 start=True, stop=True)
                o = stp.tile([F, F], dt.float16, tag=f"wsc{scaled_w.n}")
                scaled_w.n += 1
                nc.scalar.activation(o[:], psmall[:], ACTF.Copy)
                return o
            scaled_w.n = 0

            # ---------- P0: bn0 ----------
            ss0 = stp.tile([F, NCH], dt.float32, tag="ss0")
            sq0 = stp.tile([F, NCH], dt.float32, tag="sq0")
            for c in range(NCH):
                sl = slice(c * CHUNK, (c + 1) * CHUNK)
                nat = tp.tile([2, CHUNK], dt.float16, tag="nat")
                nc.sync.dma_start(nat[:], naT.ap()[:, sl])
                ps = ps_a.tile([F, CHUNK], dt.float32, tag="mm")
                nc.tensor.matmul(ps[:], lhsT=W0_sb[:], rhs=nat[:],
                                 start=True, stop=True)
                stat_chunk(ps, c, ss0, sq0)
                nc.scalar.activation(xbuf[:, sl], ps[:], ACTF.Copy)
            arbuf0 = stp.tile([F, 2], dt.float32, tag="arb0")
            nc.vector.tensor_copy(arbuf0[:, 0:1], colred(ss0)[:])
            nc.vector.tensor_copy(arbuf0[:, 1:2], colred(sq0)[:])
            rb0 = fire_ar(arbuf0, 2, "ar0")
            g_ss = afp.tile([F, 1], dt.float32)
            nc.vector.tensor_copy(g_ss[:], rb0[:, 0:1])
            g_sq = afp.tile([F, 1], dt.float32)
            nc.vector.tensor_copy(g_sq[:], rb0[:, 1:2])
            sc0, sh0, _ = fin_sums(gc_sb["g0"], gc_sb["bt0"], g_ss, g_sq)
            for c in range(NCH):
                sl = slice(c * CHUNK, (c + 1) * CHUNK)
                nc.scalar.activation(xbuf[:, sl], xbuf[:, sl], ACTF.Relu,
                                     bias=sh0[:], scale=sc0[:])
            p_col = afp.tile([F, 1], dt.float32)   # pad-node column (fp32)
            nc.scalar.activation(p_col[:], sh0[:], ACTF.Relu)

            # ---------- transpose x0 + AG#0 ----------
            def ship_table(ag_in):
                for b in range(NBLK):
                    bs = slice(b * F, (b + 1) * F)
                    pt = ps_tp.tile([F, F], dt.float16, tag="tpt")
                    nc.tensor.transpose(pt[:], xbuf[:, bs], I16_sb[:])
                    tb = tp.tile([F, F], dt.float16, tag="tb")
                    nc.vector.tensor_copy(tb[:], pt[:])
                    nc.sync.dma_start(ag_in[bs, :], tb[:])

            ship_table(ag_ins[0])
            if not DBG_NO_CC:
                nc.gpsimd.collective_compute(
                    "AllGather", OP.bypass, replica_groups=[list(range(NCORES))],
                    ins=[ag_ins[0].opt()], outs=[agos[0].opt()])

            # ---------- s-pass ----------
            for b in range(NBLK):
                pss = psmall[0:1, :]
                for k in range(cpb2):
                    ci = b * cpb2 + k
                    if ci % 8 == 0:
                        ohrt = ohrp.tile([128, 8, F], dt.float8e4, tag="ohr")
                        nc.sync.dma_start(ohrt[:], ohr.ap()[ci // 8])
                    nc.tensor.matmul(pss, lhsT=eav_sb[:, ci:ci + 1],
                                     rhs=ohrt[:, ci % 8, :],
                                     start=(k == 0), stop=(k == cpb2 - 1))
                nc.vector.tensor_copy(s_row[:, b * F:(b + 1) * F], pss)

            s_bounce = dram.tile([1, SHARD], dt.float16, tag="sbnc", name="sbnc")
            nc.sync.dma_start(s_bounce[:], s_row[:])
            nc.sync.dma_start(scv[1:2, :], s_bounce[:])
            # moments of (s, c_out) over local shard -> [1,5] partials
            momp = stp.tile([1, 8], dt.float32, tag="momp")
            nc.vector.tensor_reduce(momp[:, 0:1], s_row[:], AX, OP.add)
            nc.vector.tensor_reduce(momp[:, 1:2], cout_row[:], AX, OP.add)
            t_mom = stp.tile([1, SHARD], dt.float16, tag="tmom")
            nc.vector.tensor_mul(t_mom[:], s_row[:], cout_row[:])
            nc.vector.tensor_reduce(momp[:, 2:3], t_mom[:], AX, OP.add)
            nc.scalar.activation(t_mom[:], s_row[:], ACTF.Square,
                                 accum_out=momp[:, 3:4])
            nc.scalar.activation(t_mom[:], cout_row[:], ACTF.Square,
                                 accum_out=momp[:, 4:5])
            ones_row = stp.tile([1, F], dt.float32, tag="ones")
            nc.vector.memset(ones_row[:], 1.0)
            mom_bc = None   # [F,5] fp32 broadcast moments (set at layer 0)

            # ---------- layers ----------
            sc2 = sh2 = None
            for i in range(L):
                # --- P1: xn stats (overlaps scatter) ---
                ssn = stp.tile([F, NCH], dt.float32, tag=f"ssn{i}")
                sqn = stp.tile([F, NCH], dt.float32, tag=f"sqn{i}")
                for c in range(NCH):
                    sl = slice(c * CHUNK, (c + 1) * CHUNK)
                    ps = ps_a.tile([F, CHUNK], dt.float32, tag="mm")
                    nc.tensor.matmul(ps[:], lhsT=Wn_sb[i][:], rhs=xbuf[:, sl],
                                     start=True, stop=True)
                    stat_chunk(ps, c, ssn, sqn)
                p16 = col16(p_col)
                q_n = pad_mm(Wn_sb[i], p16)
                if i == 0:
                    arb = stp.tile([F, 7], dt.float32, tag="arb1a0")
                    nc.vector.memset(arb[:], 0.0)
                    nc.vector.tensor_copy(arb[0:1, 2:7], momp[:, 0:5])
                else:
                    arb = stp.tile([F, 2], dt.float32, tag=f"arb1a{i}")
                nc.vector.tensor_copy(arb[:, 0:1], colred(ssn)[:])
                nc.vector.tensor_copy(arb[:, 1:2], colred(sqn)[:])
                rb1a = fire_ar(arb, 7 if i == 0 else 2, f"ar1a{i}")

                # --- scatter phase ---
                if DBG_NO_SCATTER:
                    nc.vector.memset(agx[:], 0.0)
                if not DBG_NO_SCATTER:
                    for h in range(2):
                        nc.sync.dma_start(htabs[h][:],
                                          agos[i][h * HALF:(h + 1) * HALF, :])
                for h in range(2 if not DBG_NO_SCATTER else 0):
                    for call in range(NCALLS_H):
                        gt = gp.tile([128, KPC, F], dt.float16, tag="g")
                        j0 = h * nstream_h + call * GCALL
                        nc.gpsimd.dma_gather(
                            out_ap=gt[:],
                            in_ap=htabs[h][:],
                            idxs_ap=gidx_sb[:, j0 // 16:(j0 + GCALL) // 16],
                            num_idxs=GCALL, num_idxs_reg=GCALL, elem_size=F,
                            queue_num=call % 4)
                        for k8 in range(KPC):
                            ci = call * KPC + k8          # chunk within half
                            b = ci // cpb
                            k = ci % cpb
                            if k == 0:
                                psb = ps_sc.tile([F, F], dt.float32, tag="sc")
                            nc.tensor.matmul(
                                psb[:], lhsT=gt[:, k8, :],
                                rhs=ohc_sb[:, (h * nch_h + ci) * F:
                                           (h * nch_h + ci + 1) * F],
                                start=(k == 0), stop=(k == cpb - 1))
                            if k == cpb - 1:
                                dst = agx[:, b * F:(b + 1) * F]
                                if h == 0:
                                    nc.scalar.activation(dst, psb[:], ACTF.Copy)
                                else:
                                    nc.vector.scalar_tensor_tensor(
                                        out=dst, in0=psb[:], scalar=1.0,
                                        in1=dst, op0=OP.mult, op1=OP.add)

                # --- P2: aggr stats ---
                ssa = stp.tile([F, NCH], dt.float32, tag=f"ssa{i}")
                sqa = stp.tile([F, NCH], dt.float32, tag=f"sqa{i}")
                for c in range(NCH):
                    sl = slice(c * CHUNK, (c + 1) * CHUNK)
                    ps = ps_a.tile([F, CHUNK], dt.float32, tag="mm")
                    nc.tensor.matmul(ps[:], lhsT=Wb_sb[i][:], rhs=agx[:, sl],
                                     start=True, stop=False)
                    nc.tensor.matmul(ps[:], lhsT=BnbR_sb[i][:], rhs=scv[0:1, sl],
                                     start=False, stop=True)
                    stat_chunk(ps, c, ssa, sqa)
                arb_b = stp.tile([F, 2], dt.float32, tag=f"arb1b{i}")
                nc.vector.tensor_copy(arb_b[:, 0:1], colred(ssa)[:])
                nc.vector.tensor_copy(arb_b[:, 1:2], colred(sqa)[:])
                rb1b = fire_ar(arb_b, 2, f"ar1b{i}")

                # --- finalize n / a / e ---
                if i == 0:
                    nc.tensor.matmul(psmall[:, 0:5], lhsT=ones_row[:],
                                     rhs=rb1a[0:1, 2:7], start=True, stop=True)
                    mom_bc = stp.tile([F, 5], dt.float32, tag="mombc")
                    nc.vector.tensor_scalar_mul(mom_bc[:], psmall[:, 0:5], 1.0 / NREAL)
                n_ss = afp.tile([F, 1], dt.float32)
                nc.vector.tensor_copy(n_ss[:], rb1a[:, 0:1])
                n_sq = afp.tile([F, 1], dt.float32)
                nc.vector.tensor_copy(n_sq[:], rb1a[:, 1:2])
                sc_n, sh_n, _ = fin_sums(gc_sb[f"gn{i}"], gc_sb[f"btn{i}"],
                                         n_ss, n_sq, corr=q_n)
                a_ss = afp.tile([F, 1], dt.float32)
                nc.vector.tensor_copy(a_ss[:], rb1b[:, 0:1])
                a_sq = afp.tile([F, 1], dt.float32)
                nc.vector.tensor_copy(a_sq[:], rb1b[:, 1:2])
                sc_a, sh_a, _ = fin_sums(gc_sb[f"gnb{i}"], gc_sb[f"btnb{i}"],
                                         a_ss, a_sq)
                # analytic ea stats: mean = w*mu_s + b*mu_c
                # E2 = w^2*Mss + 2wb*Msc + b^2*Mcc    (mom cols: mu_s,mu_c,Msc,Mss,Mcc)
                wcol, bcol = WecC_sb[i][:, 0:1], WecC_sb[i][:, 1:2]
                me = afp.tile([F, 1], dt.float32)
                nc.vector.tensor_mul(me[:], wcol, mom_bc[:, 0:1])
                nc.vector.scalar_tensor_tensor(
                    out=me[:], in0=bcol, scalar=mom_bc[:, 1:2], in1=me[:],
                    op0=OP.mult, op1=OP.add)
                w2 = afp.tile([F, 1], dt.float32)
                nc.scalar.activation(w2[:], wcol, ACTF.Square)
                b2 = afp.tile([F, 1], dt.float32)
                nc.scalar.activation(b2[:], bcol, ACTF.Square)
                wb2 = afp.tile([F, 1], dt.float32)
                nc.vector.tensor_mul(wb2[:], wcol, bcol)
                e2 = afp.tile([F, 1], dt.float32)
                nc.vector.tensor_mul(e2[:], w2[:], mom_bc[:, 3:4])
                nc.vector.scalar_tensor_tensor(
                    out=e2[:], in0=wb2[:], scalar=mom_bc[:, 2:3], in1=e2[:],
                    op0=OP.mult, op1=OP.add)
                nc.vector.scalar_tensor_tensor(
                    out=e2[:], in0=wb2[:], scalar=mom_bc[:, 2:3], in1=e2[:],
                    op0=OP.mult, op1=OP.add)
                nc.vector.scalar_tensor_tensor(
                    out=e2[:], in0=b2[:], scalar=mom_bc[:, 4:5], in1=e2[:],
                    op0=OP.mult, op1=OP.add)
                me2 = afp.tile([F, 1], dt.float32)
                nc.scalar.activation(me2[:], me[:], ACTF.Square)
                ve = afp.tile([F, 1], dt.float32)
                nc.vector.tensor_sub(ve[:], e2[:], me2[:])
                sc_e, sh_e = fin_mv(gc_sb[f"ge{i}"], gc_sb[f"bte{i}"], me, ve)
                # combined shift; scale-folded weights; outer lhsT3
                shsum = afp.tile([F, 1], dt.float32)
                nc.vector.tensor_add(shsum[:], sh_n[:], sh_e[:])
                nc.vector.tensor_add(shsum[:], shsum[:], sh_a[:])
                Wn_sc = scaled_w(WnT_sb[i], sc_n)
                Wb_sc = scaled_w(WbT_sb[i], sc_a)
                # lhsT3 rows pair with scv rows (c_in, s, c_out):
                # (sc_a*bnb, sc_e*w_e, sc_e*b_e); assembled via SBUF DMAs
                nc.tensor.transpose(psmall[0:1, :], sc_e[:], I32_sb[:])
                sce_row = stp.tile([1, F], dt.float32, tag=f"scer{i}")
                nc.vector.tensor_copy(sce_row[:], psmall[0:1, :])
                nc.tensor.transpose(psmall[0:1, :], sc_a[:], I32_sb[:])
                sca_row = stp.tile([1, F], dt.float32, tag=f"scar{i}")
                nc.vector.tensor_copy(sca_row[:], psmall[0:1, :])
                l3r = [stp.tile([1, F], dt.float16, tag=f"l3r{k}_{i}",
                                name=f"l3r{k}_{i}") for k in range(3)]
                nc.vector.tensor_mul(l3r[0][:], BnbR_sb[i][:], sca_row[:])
                nc.vector.tensor_mul(l3r[1][:], WecA_sb[i][:], sce_row[:])
                nc.vector.tensor_mul(l3r[2][:], WecB_sb[i][:], sce_row[:])

                # --- P3: y1 + m1 ---
                ss1 = stp.tile([F, NCH], dt.float32, tag=f"ss1{i}")
                sq1 = stp.tile([F, NCH], dt.float32, tag=f"sq1{i}")
                for c in range(NCH):
                    sl = slice(c * CHUNK, (c + 1) * CHUNK)
                    ps = ps_a.tile([F, CHUNK], dt.float32, tag="mm")
                    nc.tensor.matmul(ps[:], lhsT=Wn_sc[:], rhs=xbuf[:, sl],
                                     start=True, stop=False)
                    nc.tensor.matmul(ps[:], lhsT=Wb_sc[:], rhs=agx[:, sl],
                                     start=False, stop=False)
                    nc.tensor.matmul(ps[:], lhsT=l3r[0][:], rhs=scv[0:1, sl],
                                     start=False, stop=False)
                    nc.tensor.matmul(ps[:], lhsT=l3r[1][:], rhs=s_row[:, sl],
                                     start=False, stop=False)
                    nc.tensor.matmul(ps[:], lhsT=l3r[2][:], rhs=cout_row[:, sl],
                                     start=False, stop=True)
                    y1 = tp.tile([F, CHUNK], dt.float16, tag="y1")
                    nc.scalar.activation(y1[:], ps[:], ACTF.Relu, bias=shsum[:])
                    pm = ps_b.tile([F, CHUNK], dt.float32, tag="pm")
                    nc.tensor.matmul(pm[:], lhsT=W1_sb[i][:], rhs=y1[:],
                                     start=True, stop=True)
                    stat_chunk(pm, c, ss1, sq1)
                    nc.scalar.activation(xbuf[:, sl], pm[:], ACTF.Copy)
                r1 = afp.tile([F, 1], dt.float32)
                nc.scalar.activation(r1[:], q_n[:], ACTF.Relu,
                                     bias=shsum[:], scale=sc_n[:])
                m1p = pad_mm(W1_sb[i], col16(r1))
                arb2 = stp.tile([F, 2], dt.float32, tag=f"arb2{i}")
                nc.vector.tensor_copy(arb2[:, 0:1], colred(ss1)[:])
                nc.vector.tensor_copy(arb2[:, 1:2], colred(sq1)[:])
                rb2 = fire_ar(arb2, 2, f"ar2{i}")
                m_ss = afp.tile([F, 1], dt.float32)
                nc.vector.tensor_copy(m_ss[:], rb2[:, 0:1])
                m_sq = afp.tile([F, 1], dt.float32)
                nc.vector.tensor_copy(m_sq[:], rb2[:, 1:2])
                sc1, sh1, _ = fin_sums(gc_sb[f"gm1{i}"], gc_sb[f"btm1{i}"],
                                       m_ss, m_sq, corr=m1p)

                # --- P4: y2 + m2 ---
                ss2 = stp.tile([F, NCH], dt.float32, tag=f"ss2{i}")
                sq2 = stp.tile([F, NCH], dt.float32, tag=f"sq2{i}")
                for c in range(NCH):
                    sl = slice(c * CHUNK, (c + 1) * CHUNK)
                    y2 = tp.tile([F, CHUNK], dt.float16, tag="y2")
                    nc.scalar.activation(y2[:], xbuf[:, sl], ACTF.Relu,
                                         bias=sh1[:], scale=sc1[:])
                    pm = ps_b.tile([F, CHUNK], dt.float32, tag="pm")
                    nc.tensor.matmul(pm[:], lhsT=W2_sb[i][:], rhs=y2[:],
                                     start=True, stop=True)
                    stat_chunk(pm, c, ss2, sq2)
                    nc.scalar.activation(xbuf[:, sl], pm[:], ACTF.Copy)
                y2p = afp.tile([F, 1], dt.float32)
                nc.scalar.activation(y2p[:], m1p[:], ACTF.Relu,
                                     bias=sh1[:], scale=sc1[:])
                m2p = pad_mm(W2_sb[i], col16(y2p))
                arb3 = stp.tile([F, 2], dt.float32, tag=f"arb3{i}")
                nc.vector.tensor_copy(arb3[:, 0:1], colred(ss2)[:])
                nc.vector.tensor_copy(arb3[:, 1:2], colred(sq2)[:])
                rb3 = fire_ar(arb3, 2, f"ar3{i}")
                m2ss = afp.tile([F, 1], dt.float32)
                nc.vector.tensor_copy(m2ss[:], rb3[:, 0:1])
                m2sq = afp.tile([F, 1], dt.float32)
                nc.vector.tensor_copy(m2sq[:], rb3[:, 1:2])
                sc2, sh2, _ = fin_sums(gc_sb[f"gm2{i}"], gc_sb[f"btm2{i}"],
                                       m2ss, m2sq, corr=m2p)

                # --- P5: x_next (or output) ---
                if i < L - 1:
                    for c in range(NCH):
                        sl = slice(c * CHUNK, (c + 1) * CHUNK)
                        nc.scalar.activation(xbuf[:, sl], xbuf[:, sl], ACTF.Relu,
                                             bias=sh2[:], scale=sc2[:])
                    pnew = afp.tile([F, 1], dt.float32)
                    nc.scalar.activation(pnew[:], m2p[:], ACTF.Relu,
                                         bias=sh2[:], scale=sc2[:])
                    p_col = pnew
                    ship_table(ag_ins[i + 1])
                    if not DBG_NO_CC:
                        nc.gpsimd.collective_compute(
                            "AllGather", OP.bypass,
                            replica_groups=[list(range(NCORES))],
                            ins=[ag_ins[i + 1].opt()], outs=[agos[i + 1].opt()])
                else:
                    for c in range(NCH):
                        sl = slice(c * CHUNK, (c + 1) * CHUNK)
                        of = tp.tile([F, CHUNK], dt.float32, tag="of")
                        nc.scalar.activation(of[:], xbuf[:, sl], ACTF.Relu,
                                             bias=sh2[:], scale=sc2[:])
                        nc.sync.dma_start(out.ap()[:, sl], of[:])

    nc.compile()
    return nc


def kernel(**inputs):
    import sys
    for p in ("/opt/trn_rl_repo",):
        if p not in sys.path:
            sys.path.insert(0, p)
    from concourse import bass_utils

    meta = _prep(inputs["node_attr"], inputs["edge_index"], inputs["edge_attr"])
    nc = _build(meta)

    def col(v):
        return np.ascontiguousarray(v.astype(np.float32).reshape(F, 1))

    base = dict(
        W0=inputs["W0"].astype(F16),
        I16=np.eye(F, dtype=F16),
        I32=np.eye(F, dtype=np.float32),
        g0=col(inputs["g0"]), bt0=col(inputs["bt0"]),
    )
    for i in range(L):
        base[f"Wn{i}"] = inputs["Wnode"][i].astype(F16)
        base[f"WnT{i}"] = np.ascontiguousarray(inputs["Wnode"][i].T).astype(F16)
        base[f"Wb{i}"] = inputs["Wnb"][i].astype(F16)
        base[f"WbT{i}"] = np.ascontiguousarray(inputs["Wnb"][i].T).astype(F16)
        base[f"W1{i}"] = inputs["Wm1"][i].astype(F16)
        base[f"W2{i}"] = inputs["Wm2"][i].astype(F16)
        wec = np.stack([inputs["Wedge"][i][0], inputs["bedge"][i]])
        base[f"WecA{i}"] = np.ascontiguousarray(wec[0:1].astype(np.float32))
        base[f"WecB{i}"] = np.ascontiguousarray(wec[1:2].astype(np.float32))
        base[f"WecC{i}"] = np.ascontiguousarray(wec.T.astype(np.float32))
        base[f"BnbR{i}"] = np.ascontiguousarray(
            inputs["bnb"][i].astype(F16).reshape(1, F))
        for nm in ("gn", "btn", "ge", "bte", "gnb", "btnb",
                   "gm1", "btm1", "gm2", "btm2"):
            base[f"{nm}{i}"] = col(inputs[nm][i])

    in_maps = []
    for r in range(NCORES):
        m = dict(base)
        m["naT"] = meta["naT"][r]
        m["cip"] = meta["cip"][r]
        m["gidx"] = meta["gidx"][r]
        m["ohc"] = meta["ohc"][r]
        m["ohr"] = meta["ohr"][r]
        m["eav"] = meta["eav_t"][r]
        in_maps.append(m)

    res = bass_utils.run_bass_kernel_spmd(
        nc, in_maps, core_ids=list(range(NCORES)))
    full = np.concatenate([res.results[r]["out"] for r in range(NCORES)], axis=1)
    return np.ascontiguousarray(full.T[:NREAL]).astype(np.float32)


if __name__ == "__main__":
    pass


# revision 44
# speedup vs baseline: 2.2149x; 2.2149x over previous
"""GNN message-passing kernel for Trainium2 (Bass/Tile), 8-core SPMD.

Fully sharded design (v2):
- Core r owns nodes [r*5120, (r+1)*5120); ALL dense compute is sharded 8x.
  BN batch stats are per-shard partial sums + a small AllReduce; the 960
  padding nodes (40000 -> 40960) are corrected analytically by tracking the
  (identical) pad-node column p through every transform.
- Neighbor aggregation uses matmul commutation:
      segment_sum((x@Wnb)[row], col) = segment_sum(x[row], col) @ Wnb
  so the gather table is x itself (node-major, AllGather'd once per layer);
  no per-layer h-table pass.
- Gather: dma_gather of 256B rows (int16 half-relative indices), scatter-add
  via PE one-hot matmuls (fp8 one-hots SBUF-resident, loaded once).
- Edge branch ea = segment_sum(edge_attr@Wedge + bedge, row) factors into
  rank-2 outer products of (s, c_out); its BN stats come in closed form from
  5 scalar moments of (s, c_out); s = segment_sum(edge_attr, row) is computed
  once on device by a row-sorted one-hot matmul pass and stays core-local.
- y1 = relu(bn_n(xn) + bn_a(aggr) + bn_e(ea)) is computed as a single PSUM
  accumulation: x@(Wn diag(sc_n)) + agx@(Wnb diag(sc_a)) + lhsT3-outer, with
  the scale-folded weights built on device via diag matmuls.
"""
import numpy as np
import ml_dtypes

F = 128
L = 3
EPS = 1e-5
NREAL = 40000
NE = 640000
NCORES = 8
NP_ = 40960
SHARD = NP_ // NCORES       # 5120
HALF = NP_ // 2             # 20480
NBLK = SHARD // F           # 40 blocks per core
CHUNK = 512
NCH = SHARD // CHUNK        # 10 dense chunks per core
NPAD = NP_ - NREAL          # 960

F16 = np.float16
FP8 = ml_dtypes.float8_e4m3


def _ceil(a, b):
    return -(-a // b)


def _wrap_idx16(vals):
    """int16 gather-index layout: value j at [j%16, j//16], tiled to 128 parts."""
    n = vals.shape[0]
    a = vals.reshape(n // 16, 16).T.astype(np.int16)   # [16, n/16]
    return np.tile(a, (8, 1))                          # [128, n/16]


def _prep(node_attr, edge_index, edge_attr):
    """Host-side index preprocessing -> per-core arrays + metadata."""
    row = edge_index[0].astype(np.int64)
    col = edge_index[1].astype(np.int64)
    ea = edge_attr[:, 0].astype(np.float32)

    shard = col // SHARD
    half = row // HALF
    blk = (col % SHARD) // F
    tloc = col % F

    # --- col pass (neighbor aggregation of raw x) ---
    cnt = np.zeros((NCORES, 2, NBLK), np.int64)
    np.add.at(cnt, (shard, half, blk), 1)
    sseg = _ceil(max(int(cnt.max()), 1), F) * F
    cpb = sseg // F                   # chunks per (half, block)
    nch_h = NBLK * cpb                # chunks per half
    nstream_h = NBLK * sseg           # slots per half
    nch = 2 * nch_h
    order = np.lexsort((blk, half, shard))
    so_shard, so_half, so_blk = shard[order], half[order], blk[order]
    grp = ((so_shard * 2 + so_half) * NBLK + so_blk)
    grp_start = np.zeros(NCORES * 2 * NBLK + 1, np.int64)
    np.add.at(grp_start, grp + 1, 1)
    grp_start = np.cumsum(grp_start)
    within = np.arange(NE) - grp_start[grp]
    slot = (so_half * NBLK + so_blk) * sseg + within   # slot in core stream

    gsrc = np.broadcast_to(
        (np.arange(2 * nstream_h) % HALF).astype(np.int16),
        (NCORES, 2 * nstream_h)).copy()
    gsrc[so_shard, slot] = (row[order] - so_half * HALF).astype(np.int16)
    gidx = np.stack([_wrap_idx16(gsrc[r]) for r in range(NCORES)])  # [8,128,S/16]

    # one-hot flat for SBUF residency: [core, 128 part(slot%128), nch*F]
    ohc = np.zeros((NCORES, 128, nch * F), FP8)
    ohc[so_shard, slot % F, (slot // F) * F + tloc[order]] = 1.0

    # --- row pass (s = segment_sum(edge_attr, row)) ---
    rshard = row // SHARD
    rblk = (row % SHARD) // F
    rloc = row % F
    rcnt = np.zeros((NCORES, NBLK), np.int64)
    np.add.at(rcnt, (rshard, rblk), 1)
    rseg = _ceil(max(int(rcnt.max()), 1), F) * F
    cpb2 = rseg // F
    nch2 = NBLK * cpb2
    rorder = np.lexsort((rblk, rshard))
    ro_shard, ro_blk = rshard[rorder], rblk[rorder]
    rgrp = ro_shard * NBLK + ro_blk
    rgs = np.zeros(NCORES * NBLK + 1, np.int64)
    np.add.at(rgs, rgrp + 1, 1)
    rgs = np.cumsum(rgs)
    rwithin = np.arange(NE) - rgs[rgrp]
    rslot = ro_blk * rseg + rwithin

    eav = np.zeros((NCORES, NBLK * rseg), np.float32)
    eav[ro_shard, rslot] = ea[rorder]
    ohr = np.zeros((NCORES, nch2 // 8, 128, 8, F), FP8)
    ohr[ro_shard, (rslot // F) // 8, rslot % F, (rslot // F) % 8, rloc[rorder]] = 1.0
    eav_t = np.ascontiguousarray(
        eav.reshape(NCORES, nch2, F).transpose(0, 2, 1)).astype(F16)

    # degree counts (pure edge_index metadata), per-core shard slices
    c_out = np.bincount(row, minlength=NP_).astype(np.float32)
    c_in = np.bincount(col, minlength=NP_).astype(np.float32)
    cip = np.stack([c_in, c_out]).reshape(2, NCORES, SHARD).transpose(1, 0, 2)
    cip = np.ascontiguousarray(cip).astype(F16)   # [8, 2=(c_in,c_out), SHARD]

    naT = np.zeros((2, NP_), np.float32)
    naT[:, :NREAL] = node_attr.T
    naT = np.ascontiguousarray(
        naT.reshape(2, NCORES, SHARD).transpose(1, 0, 2)).astype(F16)  # [8,2,SHARD]

    return dict(sseg=sseg, cpb=cpb, nch=nch, nch_h=nch_h, nstream_h=nstream_h,
                rseg=rseg, cpb2=cpb2, nch2=nch2,
                gidx=gidx, ohc=ohc, ohr=ohr, eav_t=eav_t,
                cip=cip, naT=naT)


def _build(meta):
    """Build the Bass program."""
    import os
    DBG_NO_SCATTER = bool(int(os.environ.get("K_NO_SCATTER", "0")))
    DBG_NO_CC = bool(int(os.environ.get("K_NO_CC", "0")))
    import concourse.bass as bass
    import concourse.tile as tile
    from concourse import bacc, mybir

    sseg, cpb, nch_h = meta["sseg"], meta["cpb"], meta["nch_h"]
    nstream_h = meta["nstream_h"]
    nch = meta["nch"]
    cpb2, nch2 = meta["cpb2"], meta["nch2"]
    GCALL = 1024                      # 64 descriptors/engine = one packet
    NCALLS_H = nstream_h // GCALL     # = 5*cpb (45 for cpb=9)
    KPC = GCALL // F                  # chunks per gather call = 8
    dt = mybir.dt
    AX = mybir.AxisListType.X
    OP = mybir.AluOpType
    ACTF = mybir.ActivationFunctionType

    nc = bacc.Bacc("TRN2", target_bir_lowering=False, debug=False,
                   num_devices=NCORES, num_swdge_queues=4)

    def din(name, shape, d):
        return nc.dram_tensor(name, shape, d, kind="ExternalInput")

    naT = din("naT", [2, SHARD], dt.float16)
    cip = din("cip", [2, SHARD], dt.float16)
    gidx = din("gidx", [128, 2 * nstream_h // 16], dt.int16)
    ohc = din("ohc", [128, nch * F], dt.float8e4)
    ohr = din("ohr", [nch2 // 8, 128, 8, F], dt.float8e4)
    eav = din("eav", [128, nch2], dt.float16)
    W0 = din("W0", [2, F], dt.float16)
    Wn = [din(f"Wn{i}", [F, F], dt.float16) for i in range(L)]
    WnT = [din(f"WnT{i}", [F, F], dt.float16) for i in range(L)]
    Wb = [din(f"Wb{i}", [F, F], dt.float16) for i in range(L)]
    WbT = [din(f"WbT{i}", [F, F], dt.float16) for i in range(L)]
    W1 = [din(f"W1{i}", [F, F], dt.float16) for i in range(L)]
    W2 = [din(f"W2{i}", [F, F], dt.float16) for i in range(L)]
    WecA = [din(f"WecA{i}", [1, F], dt.float32) for i in range(L)]    # w_e row
    WecB = [din(f"WecB{i}", [1, F], dt.float32) for i in range(L)]    # b_e row
    WecC = [din(f"WecC{i}", [F, 2], dt.float32) for i in range(L)]    # cols
    BnbR = [din(f"BnbR{i}", [1, F], dt.float16) for i in range(L)]    # bnb row
    I16 = din("I16", [F, F], dt.float16)
    I32 = din("I32", [F, F], dt.float32)
    gcol = {}
    for nm in ("g0", "bt0"):
        gcol[nm] = din(nm, [F, 1], dt.float32)
    for i in range(L):
        for nm in ("gn", "btn", "ge", "bte", "gnb", "btnb",
                   "gm1", "btm1", "gm2", "btm2"):
            gcol[f"{nm}{i}"] = din(f"{nm}{i}", [F, 1], dt.float32)

    out = nc.dram_tensor("out", [F, SHARD], dt.float32, kind="ExternalOutput")

    with tile.TileContext(nc) as tc:
        import contextlib
        ctx = contextlib.ExitStack()
        with ctx:
            sb = ctx.enter_context(tc.tile_pool(name="sb", bufs=1))
            wpool = ctx.enter_context(tc.tile_pool(name="wp", bufs=1))
            tp = ctx.enter_context(tc.tile_pool(name="tp", bufs=2))
            gp = ctx.enter_context(tc.tile_pool(name="gp", bufs=2))
            ohrp = ctx.enter_context(tc.tile_pool(name="ohrp", bufs=2))
            stp = ctx.enter_context(tc.tile_pool(name="stp", bufs=1))
            afp = ctx.enter_context(tc.tile_pool(name="afp", bufs=4))
            ps_a = ctx.enter_context(tc.tile_pool(name="psa", bufs=2, space="PSUM"))
            ps_b = ctx.enter_context(tc.tile_pool(name="psb", bufs=2, space="PSUM"))
            ps_sc = ctx.enter_context(tc.tile_pool(name="pssc", bufs=2, space="PSUM"))
            ps_sm = ctx.enter_context(tc.tile_pool(name="pssm", bufs=1, space="PSUM"))
            ps_tp = ctx.enter_context(tc.tile_pool(name="pstp", bufs=1, space="PSUM"))
            dram = ctx.enter_context(tc.tile_pool(name="dram", bufs=1, space="DRAM"))

            gsems = [nc.alloc_semaphore(f"gsem{q}") for q in range(4)]
            # ---- persistent SBUF ----
            xbuf = sb.tile([F, SHARD], dt.float16)
            agx = sb.tile([F, SHARD], dt.float16)
            scv = sb.tile([3, SHARD], dt.float16)      # rows: c_in, s, c_out
            nc.sync.dma_start(scv[0:1, :], cip.ap()[0:1, :])
            nc.sync.dma_start(scv[2:3, :], cip.ap()[1:2, :])
            s_row = sb.tile([1, SHARD], dt.float16)
            cout_row = sb.tile([1, SHARD], dt.float16)
            nc.sync.dma_start(cout_row[:], cip.ap()[1:2, :])
            gidx_sb = sb.tile([128, 2 * nstream_h // 16], dt.int16)
            nc.sync.dma_start(gidx_sb[:], gidx.ap())
            eav_sb = sb.tile([128, nch2], dt.float16)
            nc.sync.dma_start(eav_sb[:], eav.ap())
            trash = sb.tile([F, CHUNK], dt.float32)
            psmall = ps_sm.tile([F, F], dt.float32, tag="small")

            def wload(t_, tag):
                w = wpool.tile(list(t_.shape), t_.dtype, tag=tag)
                nc.sync.dma_start(w[:], t_.ap())
                return w

            W0_sb = wload(W0, "w0")
            I16_sb = wload(I16, "i16")
            I32_sb = wload(I32, "i32")
            Wn_sb = [wload(Wn[i], f"wn{i}") for i in range(L)]
            WnT_sb = [wload(WnT[i], f"wnt{i}") for i in range(L)]
            Wb_sb = [wload(Wb[i], f"wb{i}") for i in range(L)]
            WbT_sb = [wload(WbT[i], f"wbt{i}") for i in range(L)]
            W1_sb = [wload(W1[i], f"w1{i}") for i in range(L)]
            W2_sb = [wload(W2[i], f"w2{i}") for i in range(L)]
            WecA_sb = [wload(WecA[i], f"weca{i}") for i in range(L)]
            WecB_sb = [wload(WecB[i], f"wecb{i}") for i in range(L)]
            WecC_sb = [wload(WecC[i], f"wecc{i}") for i in range(L)]
            BnbR_sb = [wload(BnbR[i], f"bnbr{i}") for i in range(L)]
            gc_sb = {nm: wload(t_, f"p{nm}") for nm, t_ in gcol.items()}

            # ---- DRAM scratch ----
            ag_ins = [dram.tile([SHARD, F], dt.float16, tag=f"agi{i}",
                                name=f"agi{i}") for i in range(L)]
            agos = [dram.tile([NP_, F], dt.float16, addr_space="Shared",
                              tag=f"ago{i}", name=f"ago{i}") for i in range(L)]
            htabs = [dram.tile([HALF, F], dt.float16, tag=f"htab{h}",
                               name=f"htab{h}") for h in range(2)]
            ar_ins, ar_outs = [], []

            def make_ar(ncols, tag):
                i_ = dram.tile([F, ncols], dt.float32, tag=f"ari{tag}",
                               name=f"ari{tag}")
                o_ = dram.tile([F, ncols], dt.float32, addr_space="Shared",
                               tag=f"aro{tag}", name=f"aro{tag}")
                return i_, o_

            # ---------- helpers ----------
            def fire_ar(buf, ncols, tag):
                if DBG_NO_CC:
                    rb = stp.tile([F, ncols], dt.float32, tag=f"rb{tag}")
                    nc.vector.tensor_scalar_mul(rb[:], buf[:, :ncols],
                                                float(NCORES))
                    return rb
                ari, aro = make_ar(ncols, tag)
                nc.gpsimd.dma_start(ari[:], buf[:, :ncols])
                nc.gpsimd.collective_compute(
                    "AllReduce", OP.add, replica_groups=[list(range(NCORES))],
                    ins=[ari.opt()], outs=[aro.opt()])
                rb = stp.tile([F, ncols], dt.float32, tag=f"rb{tag}")
                nc.sync.dma_start(rb[:], aro[:])
                return rb

            def fin_mv(g, bt, mean, var):
                """(scale, shift) from mean/var columns [F,1]."""
                v2 = afp.tile([F, 1], dt.float32)
                nc.vector.tensor_scalar_add(v2[:], var[:], EPS)
                lnv = afp.tile([F, 1], dt.float32)
                nc.scalar.activation(lnv[:], v2[:], ACTF.Ln)
                isig = afp.tile([F, 1], dt.float32)
                nc.scalar.activation(isig[:], lnv[:], ACTF.Exp, scale=-0.5)
                scale = afp.tile([F, 1], dt.float32)
                nc.vector.tensor_mul(scale[:], g[:], isig[:])
                nscale = afp.tile([F, 1], dt.float32)
                nc.vector.tensor_scalar_mul(nscale[:], scale[:], -1.0)
                shift = afp.tile([F, 1], dt.float32)
                nc.vector.scalar_tensor_tensor(
                    out=shift[:], in0=mean[:], scalar=nscale[:], in1=bt[:],
                    op0=OP.mult, op1=OP.add)
                return scale, shift

            def fin_sums(g, bt, ssum, ssq, corr=None):
                """(scale, shift) from global sum/sumsq [F,1]; corr = pad col."""
                if corr is not None:
                    c2 = afp.tile([F, 1], dt.float32)
                    nc.scalar.activation(c2[:], corr[:], ACTF.Square)
                    nc.vector.scalar_tensor_tensor(
                        out=ssum[:], in0=corr[:], scalar=-float(NPAD), in1=ssum[:],
                        op0=OP.mult, op1=OP.add)
                    nc.vector.scalar_tensor_tensor(
                        out=ssq[:], in0=c2[:], scalar=-float(NPAD), in1=ssq[:],
                        op0=OP.mult, op1=OP.add)
                mean = afp.tile([F, 1], dt.float32)
                nc.vector.tensor_scalar_mul(mean[:], ssum[:], 1.0 / NREAL)
                m2t = afp.tile([F, 1], dt.float32)
                nc.scalar.activation(m2t[:], mean[:], ACTF.Square)
                var = afp.tile([F, 1], dt.float32)
                nc.vector.scalar_tensor_tensor(
                    out=var[:], in0=ssq[:], scalar=1.0 / NREAL, in1=m2t[:],
                    op0=OP.mult, op1=OP.subtract)
                sc, sh = fin_mv(g, bt, mean, var)
                return sc, sh, mean

            def stat_chunk(ps, c, ss, sq):
                nc.vector.tensor_reduce(ss[:, c:c + 1], ps[:, :], AX, OP.add)
                nc.scalar.activation(trash[:], ps[:, :], ACTF.Square,
                                     accum_out=sq[:, c:c + 1])

            def colred(sl):
                r = afp.tile([F, 1], dt.float32)
                nc.vector.tensor_reduce(r[:], sl[:], AX, OP.add)
                return r

            def col16(col):
                t = afp.tile([F, 1], dt.float16)
                nc.vector.tensor_copy(t[:], col[:])
                return t

            def pad_mm(w_sb, col_f16):
                """[F,1] = w.T @ col via PE; returns fp32 sbuf col."""
                nc.tensor.matmul(psmall[:, 0:1], lhsT=w_sb[:], rhs=col_f16[:],
                                 start=True, stop=True)
                o = afp.tile([F, 1], dt.float32)
                nc.vector.tensor_copy(o[:], psmall[:, 0:1])
                return o

            def scaled_w(wT_sb, sc):
                """W*diag(sc) as fp16 SBUF tile, via diag matmul."""
                dg = tp.tile([F, F], dt.float16, tag="diag")
                nc.vector.tensor_scalar_mul(dg[:], I32_sb[:], sc[:])
                nc.tensor.matmul(psmall[:], lhsT=wT_sb[:], rhs=dg[:],
                                 start=True, stop=True)
                o = stp.tile([F, F], dt.float16, tag=f"wsc{scaled_w.n}")
                scaled_w.n += 1
                nc.scalar.activation(o[:], psmall[:], ACTF.Copy)
                return o
            scaled_w.n = 0

            # ---------- P0: bn0 ----------
            ss0 = stp.tile([F, NCH], dt.float32, tag="ss0")
            sq0 = stp.tile([F, NCH], dt.float32, tag="sq0")
            for c in range(NCH):
                sl = slice(c * CHUNK, (c + 1) * CHUNK)
                nat = tp.tile([2, CHUNK], dt.float16, tag="nat")
                nc.sync.dma_start(nat[:], naT.ap()[:, sl])
                ps = ps_a.tile([F, CHUNK], dt.float32, tag="mm")
                nc.tensor.matmul(ps[:], lhsT=W0_sb[:], rhs=nat[:],
                                 start=True, stop=True)
                stat_chunk(ps, c, ss0, sq0)
                nc.scalar.activation(xbuf[:, sl], ps[:], ACTF.Copy)
            arbuf0 = stp.tile([F, 2], dt.float32, tag="arb0")
            nc.vector.tensor_copy(arbuf0[:, 0:1], colred(ss0)[:])
            nc.vector.tensor_copy(arbuf0[:, 1:2], colred(sq0)[:])
            rb0 = fire_ar(arbuf0, 2, "ar0")
            g_ss = afp.tile([F, 1], dt.float32)
            nc.vector.tensor_copy(g_ss[:], rb0[:, 0:1])
            g_sq = afp.tile([F, 1], dt.float32)
            nc.vector.tensor_copy(g_sq[:], rb0[:, 1:2])
            sc0, sh0, _ = fin_sums(gc_sb["g0"], gc_sb["bt0"], g_ss, g_sq)
            for c in range(NCH):
                sl = slice(c * CHUNK, (c + 1) * CHUNK)
                nc.scalar.activation(xbuf[:, sl], xbuf[:, sl], ACTF.Relu,
                                     bias=sh0[:], scale=sc0[:])
            p_col = afp.tile([F, 1], dt.float32)   # pad-node column (fp32)
            nc.scalar.activation(p_col[:], sh0[:], ACTF.Relu)

            # ---------- transpose x0 + AG#0 ----------
            def ship_table(ag_in):
                for b in range(NBLK):
                    bs = slice(b * F, (b + 1) * F)
                    pt = ps_tp.tile([F, F], dt.float16, tag="tpt")
                    nc.tensor.transpose(pt[:], xbuf[:, bs], I16_sb[:])
                    tb = tp.tile([F, F], dt.float16, tag="tb")
                    nc.vector.tensor_copy(tb[:], pt[:])
                    nc.sync.dma_start(ag_in[bs, :], tb[:])

            ship_table(ag_ins[0])
            if not DBG_NO_CC:
                nc.gpsimd.collective_compute(
                    "AllGather", OP.bypass, replica_groups=[list(range(NCORES))],
                    ins=[ag_ins[0].opt()], outs=[agos[0].opt()])

            # ---------- s-pass ----------
            for b in range(NBLK):
                pss = psmall[0:1, :]
                for k in range(cpb2):
                    ci = b * cpb2 + k
                    if ci % 8 == 0:
                        ohrt = ohrp.tile([128, 8, F], dt.float8e4, tag="ohr")
                        nc.sync.dma_start(ohrt[:], ohr.ap()[ci // 8])
                    nc.tensor.matmul(pss, lhsT=eav_sb[:, ci:ci + 1],
                                     rhs=ohrt[:, ci % 8, :],
                                     start=(k == 0), stop=(k == cpb2 - 1))
                nc.vector.tensor_copy(s_row[:, b * F:(b + 1) * F], pss)

            s_bounce = dram.tile([1, SHARD], dt.float16, tag="sbnc", name="sbnc")
            nc.sync.dma_start(s_bounce[:], s_row[:])
            nc.sync.dma_start(scv[1:2, :], s_bounce[:])
            # moments of (s, c_out) over local shard -> [1,5] partials
            momp = stp.tile([1, 8], dt.float32, tag="momp")
            nc.vector.tensor_reduce(momp[:, 0:1], s_row[:], AX, OP.add)
            nc.vector.tensor_reduce(momp[:, 1:2], cout_row[:], AX, OP.add)
            t_mom = stp.tile([1, SHARD], dt.float16, tag="tmom")
            nc.vector.tensor_mul(t_mom[:], s_row[:], cout_row[:])
            nc.vector.tensor_reduce(momp[:, 2:3], t_mom[:], AX, OP.add)
            nc.scalar.activation(t_mom[:], s_row[:], ACTF.Square,
                                 accum_out=momp[:, 3:4])
            nc.scalar.activation(t_mom[:], cout_row[:], ACTF.Square,
                                 accum_out=momp[:, 4:5])
            ones_row = stp.tile([1, F], dt.float32, tag="ones")
            nc.vector.memset(ones_row[:], 1.0)
            mom_bc = None   # [F,5] fp32 broadcast moments (set at layer 0)

            # ---------- layers ----------
            sc2 = sh2 = None
            for i in range(L):
                # --- P1: xn stats (overlaps scatter) ---
                ssn = stp.tile([F, NCH], dt.float32, tag=f"ssn{i}")
                sqn = stp.tile([F, NCH], dt.float32, tag=f"sqn{i}")
                for c in range(NCH):
                    sl = slice(c * CHUNK, (c + 1) * CHUNK)
                    ps = ps_a.tile([F, CHUNK], dt.float32, tag="mm")
                    nc.tensor.matmul(ps[:], lhsT=Wn_sb[i][:], rhs=xbuf[:, sl],
                                     start=True, stop=True)
                    stat_chunk(ps, c, ssn, sqn)
                p16 = col16(p_col)
                q_n = pad_mm(Wn_sb[i], p16)

                # --- scatter phase ---
                if DBG_NO_SCATTER:
                    nc.vector.memset(agx[:], 0.0)
                for h in range(2 if not DBG_NO_SCATTER else 0):
                    for call in range(NCALLS_H):
                        gt = gp.tile([128, KPC, F], dt.float16, tag="g")
                        j0 = h * nstream_h + call * GCALL
                        nc.gpsimd.dma_gather(
                            out_ap=gt[:],
                            in_ap=agos[i][h * HALF:(h + 1) * HALF, :],
                            idxs_ap=gidx_sb[:, j0 // 16:(j0 + GCALL) // 16],
                            num_idxs=GCALL, num_idxs_reg=GCALL, elem_size=F,
                            prepare_only=True, sem=gsems[call % 4],
                            queue_num=call % 4)
                        nc.gpsimd.trigger_dma(count=None, queue_num=call % 4)
                        for k8 in range(KPC):
                            ci = call * KPC + k8          # chunk within half
                            b = ci // cpb
                            k = ci % cpb
                            if k == 0:
                                psb = ps_sc.tile([F, F], dt.float32, tag="sc")
                            nc.tensor.matmul(
                                psb[:], lhsT=gt[:, k8, :],
                                rhs=ohc_sb[:, (h * nch_h + ci) * F:
                                           (h * nch_h + ci + 1) * F],
                                start=(k == 0), stop=(k == cpb - 1))
                            if k == cpb - 1:
                                dst = agx[:, b * F:(b + 1) * F]
                                if h == 0:
                                    nc.scalar.activation(dst, psb[:], ACTF.Copy)
                                else:
                                    nc.vector.scalar_tensor_tensor(
                                        out=dst, in0=psb[:], scalar=1.0,
                                        in1=dst, op0=OP.mult, op1=OP.add)

                # --- P2: aggr stats ---
                ssa = stp.tile([F, NCH], dt.float32, tag=f"ssa{i}")
                sqa = stp.tile([F, NCH], dt.float32, tag=f"sqa{i}")
                for c in range(NCH):
                    sl = slice(c * CHUNK, (c + 1) * CHUNK)
                    ps = ps_a.tile([F, CHUNK], dt.float32, tag="mm")
                    nc.tensor.matmul(ps[:], lhsT=Wb_sb[i][:], rhs=agx[:, sl],
                                     start=True, stop=False)
                    nc.tensor.matmul(ps[:], lhsT=BnbR_sb[i][:], rhs=scv[0:1, sl],
                                     start=False, stop=True)
                    stat_chunk(ps, c, ssa, sqa)
                if i == 0:
                    arb = stp.tile([F, 9], dt.float32, tag="arb1a0")
                    nc.vector.memset(arb[:], 0.0)
                    nc.vector.tensor_copy(arb[0:1, 4:9], momp[:, 0:5])
                else:
                    arb = stp.tile([F, 4], dt.float32, tag=f"arb1a{i}")
                nc.vector.tensor_copy(arb[:, 0:1], colred(ssn)[:])
                nc.vector.tensor_copy(arb[:, 1:2], colred(sqn)[:])
                nc.vector.tensor_copy(arb[:, 2:3], colred(ssa)[:])
                nc.vector.tensor_copy(arb[:, 3:4], colred(sqa)[:])
                rb1a = fire_ar(arb, 9 if i == 0 else 4, f"ar1a{i}")
                rb1b = rb1a

                # --- finalize n / a / e ---
                if i == 0:
                    nc.tensor.matmul(psmall[:, 0:5], lhsT=ones_row[:],
                                     rhs=rb1a[0:1, 4:9], start=True, stop=True)
                    mom_bc = stp.tile([F, 5], dt.float32, tag="mombc")
                    nc.vector.tensor_scalar_mul(mom_bc[:], psmall[:, 0:5], 1.0 / NREAL)
                n_ss = afp.tile([F, 1], dt.float32)
                nc.vector.tensor_copy(n_ss[:], rb1a[:, 0:1])
                n_sq = afp.tile([F, 1], dt.float32)
                nc.vector.tensor_copy(n_sq[:], rb1a[:, 1:2])
                sc_n, sh_n, _ = fin_sums(gc_sb[f"gn{i}"], gc_sb[f"btn{i}"],
                                         n_ss, n_sq, corr=q_n)
                a_ss = afp.tile([F, 1], dt.float32)
                nc.vector.tensor_copy(a_ss[:], rb1b[:, 2:3])
                a_sq = afp.tile([F, 1], dt.float32)
                nc.vector.tensor_copy(a_sq[:], rb1b[:, 3:4])
                sc_a, sh_a, _ = fin_sums(gc_sb[f"gnb{i}"], gc_sb[f"btnb{i}"],
                                         a_ss, a_sq)
                # analytic ea stats: mean = w*mu_s + b*mu_c
                # E2 = w^2*Mss + 2wb*Msc + b^2*Mcc    (mom cols: mu_s,mu_c,Msc,Mss,Mcc)
                wcol, bcol = WecC_sb[i][:, 0:1], WecC_sb[i][:, 1:2]
                me = afp.tile([F, 1], dt.float32)
                nc.vector.tensor_mul(me[:], wcol, mom_bc[:, 0:1])
                nc.vector.scalar_tensor_tensor(
                    out=me[:], in0=bcol, scalar=mom_bc[:, 1:2], in1=me[:],
                    op0=OP.mult, op1=OP.add)
                w2 = afp.tile([F, 1], dt.float32)
                nc.scalar.activation(w2[:], wcol, ACTF.Square)
                b2 = afp.tile([F, 1], dt.float32)
                nc.scalar.activation(b2[:], bcol, ACTF.Square)
                wb2 = afp.tile([F, 1], dt.float32)
                nc.vector.tensor_mul(wb2[:], wcol, bcol)
                e2 = afp.tile([F, 1], dt.float32)
                nc.vector.tensor_mul(e2[:], w2[:], mom_bc[:, 3:4])
                nc.vector.scalar_tensor_tensor(
                    out=e2[:], in0=wb2[:], scalar=mom_bc[:, 2:3], in1=e2[:],
                    op0=OP.mult, op1=OP.add)
                nc.vector.scalar_tensor_tensor(
                    out=e2[:], in0=wb2[:], scalar=mom_bc[:, 2:3], in1=e2[:],
                    op0=OP.mult, op1=OP.add)
                nc.vector.scalar_tensor_tensor(
                    out=e2[:], in0=b2[:], scalar=mom_bc[:, 4:5], in1=e2[:],
                    op0=OP.mult, op1=OP.add)
                me2 = afp.tile([F, 1], dt.float32)
                nc.scalar.activation(me2[:], me[:], ACTF.Square)
                ve = afp.tile([F, 1], dt.float32)
                nc.vector.tensor_sub(ve[:], e2[:], me2[:])
                sc_e, sh_e = fin_mv(gc_sb[f"ge{i}"], gc_sb[f"bte{i}"], me, ve)
                # combined shift; scale-folded weights; outer lhsT3
                shsum = afp.tile([F, 1], dt.float32)
                nc.vector.tensor_add(shsum[:], sh_n[:], sh_e[:])
                nc.vector.tensor_add(shsum[:], shsum[:], sh_a[:])
                Wn_sc = scaled_w(WnT_sb[i], sc_n)
                Wb_sc = scaled_w(WbT_sb[i], sc_a)
                # lhsT3 rows pair with scv rows (c_in, s, c_out):
                # (sc_a*bnb, sc_e*w_e, sc_e*b_e); assembled via SBUF DMAs
                nc.tensor.transpose(psmall[0:1, :], sc_e[:], I32_sb[:])
                sce_row = stp.tile([1, F], dt.float32, tag=f"scer{i}")
                nc.vector.tensor_copy(sce_row[:], psmall[0:1, :])
                nc.tensor.transpose(psmall[0:1, :], sc_a[:], I32_sb[:])
                sca_row = stp.tile([1, F], dt.float32, tag=f"scar{i}")
                nc.vector.tensor_copy(sca_row[:], psmall[0:1, :])
                l3r = [stp.tile([1, F], dt.float16, tag=f"l3r{k}_{i}",
                                name=f"l3r{k}_{i}") for k in range(3)]
                nc.vector.tensor_mul(l3r[0][:], BnbR_sb[i][:], sca_row[:])
                nc.vector.tensor_mul(l3r[1][:], WecA_sb[i][:], sce_row[:])
                nc.vector.tensor_mul(l3r[2][:], WecB_sb[i][:], sce_row[:])

                # --- P3: y1 + m1 ---
                ss1 = stp.tile([F, NCH], dt.float32, tag=f"ss1{i}")
                sq1 = stp.tile([F, NCH], dt.float32, tag=f"sq1{i}")
                for c in range(NCH):
                    sl = slice(c * CHUNK, (c + 1) * CHUNK)
                    ps = ps_a.tile([F, CHUNK], dt.float32, tag="mm")
                    nc.tensor.matmul(ps[:], lhsT=Wn_sc[:], rhs=xbuf[:, sl],
                                     start=True, stop=False)
                    nc.tensor.matmul(ps[:], lhsT=Wb_sc[:], rhs=agx[:, sl],
                                     start=False, stop=False)
                    nc.tensor.matmul(ps[:], lhsT=l3r[0][:], rhs=scv[0:1, sl],
                                     start=False, stop=False)
                    nc.tensor.matmul(ps[:], lhsT=l3r[1][:], rhs=s_row[:, sl],
                                     start=False, stop=False)
                    nc.tensor.matmul(ps[:], lhsT=l3r[2][:], rhs=cout_row[:, sl],
                                     start=False, stop=True)
                    y1 = tp.tile([F, CHUNK], dt.float16, tag="y1")
                    nc.scalar.activation(y1[:], ps[:], ACTF.Relu, bias=shsum[:])
                    pm = ps_b.tile([F, CHUNK], dt.float32, tag="pm")
                    nc.tensor.matmul(pm[:], lhsT=W1_sb[i][:], rhs=y1[:],
                                     start=True, stop=True)
                    stat_chunk(pm, c, ss1, sq1)
                    nc.scalar.activation(xbuf[:, sl], pm[:], ACTF.Copy)
                r1 = afp.tile([F, 1], dt.float32)
                nc.scalar.activation(r1[:], q_n[:], ACTF.Relu,
                                     bias=shsum[:], scale=sc_n[:])
                m1p = pad_mm(W1_sb[i], col16(r1))
                arb2 = stp.tile([F, 2], dt.float32, tag=f"arb2{i}")
                nc.vector.tensor_copy(arb2[:, 0:1], colred(ss1)[:])
                nc.vector.tensor_copy(arb2[:, 1:2], colred(sq1)[:])
                rb2 = fire_ar(arb2, 2, f"ar2{i}")
                m_ss = afp.tile([F, 1], dt.float32)
                nc.vector.tensor_copy(m_ss[:], rb2[:, 0:1])
                m_sq = afp.tile([F, 1], dt.float32)
                nc.vector.tensor_copy(m_sq[:], rb2[:, 1:2])
                sc1, sh1, _ = fin_sums(gc_sb[f"gm1{i}"], gc_sb[f"btm1{i}"],
                                       m_ss, m_sq, corr=m1p)

                # --- P4: y2 + m2 ---
                ss2 = stp.tile([F, NCH], dt.float32, tag=f"ss2{i}")
                sq2 = stp.tile([F, NCH], dt.float32, tag=f"sq2{i}")
                for c in range(NCH):
                    sl = slice(c * CHUNK, (c + 1) * CHUNK)
                    y2 = tp.tile([F, CHUNK], dt.float16, tag="y2")
                    nc.scalar.activation(y2[:], xbuf[:, sl], ACTF.Relu,
                                         bias=sh1[:], scale=sc1[:])
                    pm = ps_b.tile([F, CHUNK], dt.float32, tag="pm")
                    nc.tensor.matmul(pm[:], lhsT=W2_sb[i][:], rhs=y2[:],
                                     start=True, stop=True)
                    stat_chunk(pm, c, ss2, sq2)
                    nc.scalar.activation(xbuf[:, sl], pm[:], ACTF.Copy)
                y2p = afp.tile([F, 1], dt.float32)
                nc.scalar.activation(y2p[:], m1p[:], ACTF.Relu,
                                     bias=sh1[:], scale=sc1[:])
                m2p = pad_mm(W2_sb[i], col16(y2p))
                arb3 = stp.tile([F, 2], dt.float32, tag=f"arb3{i}")
                nc.vector.tensor_copy(arb3[:, 0:1], colred(ss2)[:])
                nc.vector.tensor_copy(arb3[:, 1:2], colred(sq2)[:])
                rb3 = fire_ar(arb3, 2, f"ar3{i}")
                m2ss = afp.tile([F, 1], dt.float32)
                nc.vector.tensor_copy(m2ss[:], rb3[:, 0:1])
                m2sq = afp.tile([F, 1], dt.float32)
                nc.vector.tensor_copy(m2sq[:], rb3[:, 1:2])
                sc2, sh2, _ = fin_sums(gc_sb[f"gm2{i}"], gc_sb[f"btm2{i}"],
                                       m2ss, m2sq, corr=m2p)

                # --- P5: x_next (or output) ---
                if i < L - 1:
                    for c in range(NCH):
                        sl = slice(c * CHUNK, (c + 1) * CHUNK)
                        nc.scalar.activation(xbuf[:, sl], xbuf[:, sl], ACTF.Relu,
                                             bias=sh2[:], scale=sc2[:])
                    pnew = afp.tile([F, 1], dt.float32)
                    nc.scalar.activation(pnew[:], m2p[:], ACTF.Relu,
                                         bias=sh2[:], scale=sc2[:])
                    p_col = pnew
                    ship_table(ag_ins[i + 1])
                    if not DBG_NO_CC:
                        nc.gpsimd.collective_compute(
                            "AllGather", OP.bypass,
                            replica_groups=[list(range(NCORES))],
                            ins=[ag_ins[i + 1].opt()], outs=[agos[i + 1].opt()])
                else:
                    for c in range(NCH):
                        sl = slice(c * CHUNK, (c + 1) * CHUNK)
                        of = tp.tile([F, CHUNK], dt.float32, tag="of")
                        nc.scalar.activation(of[:], xbuf[:, sl], ACTF.Relu,
                                             bias=sh2[:], scale=sc2[:])
                        nc.sync.dma_start(out.ap()[:, sl], of[:])

    nc.compile()
    return nc


def kernel(**inputs):
    import sys
    for p in ("/opt/trn_rl_repo",):
        if p not in sys.path:
            sys.path.insert(0, p)
    from concourse import bass_utils

    meta = _prep(inputs["node_attr"], inputs["edge_index"], inputs["edge_attr"])
    nc = _build(meta)

    def col(v):
        return np.ascontiguousarray(v.astype(np.float32).reshape(F, 1))

    base = dict(
        W0=inputs["W0"].astype(F16),
        I16=np.eye(F, dtype=F16),
        I32=np.eye(F, dtype=np.float32),
        g0=col(inputs["g0"]), bt0=col(inputs["bt0"]),
    )
    for i in range(L):
        base[f"Wn{i}"] = inputs["Wnode"][i].astype(F16)
        base[f"WnT{i}"] = np.ascontiguousarray(inputs["Wnode"][i].T).astype(F16)
        base[f"Wb{i}"] = inputs["Wnb"][i].astype(F16)
        base[f"WbT{i}"] = np.ascontiguousarray(inputs["Wnb"][i].T).astype(F16)
        base[f"W1{i}"] = inputs["Wm1"][i].astype(F16)
        base[f"W2{i}"] = inputs["Wm2"][i].astype(F16)
        wec = np.stack([inputs["Wedge"][i][0], inputs["bedge"][i]])
        base[f"WecA{i}"] = np.ascontiguousarray(wec[0:1].astype(np.float32))
        base[f"WecB{i}"] = np.ascontiguousarray(wec[1:2].astype(np.float32))
        base[f"WecC{i}"] = np.ascontiguousarray(wec.T.astype(np.float32))
        base[f"BnbR{i}"] = np.ascontiguousarray(
            inputs["bnb"][i].astype(F16).reshape(1, F))
        for nm in ("gn", "btn", "ge", "bte", "gnb", "btnb",
                   "gm1", "btm1", "gm2", "btm2"):
            base[f"{nm}{i}"] = col(inputs[nm][i])

    in_maps = []
    for r in range(NCORES):
        m = dict(base)
        m["naT"] = meta["naT"][r]
        m["cip"] = meta["cip"][r]
        m["gidx"] = meta["gidx"][r]
        m["ohc"] = meta["ohc"][r]
        m["ohr"] = meta["ohr"][r]
        m["eav"] = meta["eav_t"][r]
        in_maps.append(m)

    res = bass_utils.run_bass_kernel_spmd(
        nc, in_maps, core_ids=list(range(NCORES)))
    full = np.concatenate([res.results[r]["out"] for r in range(NCORES)], axis=1)
    return np.ascontiguousarray(full.T[:NREAL]).astype(np.float32)


if __name__ == "__main__":
    pass


# revision 46
# speedup vs baseline: 2.2226x; 1.0035x over previous
"""GNN message-passing kernel for Trainium2 (Bass/Tile), 8-core SPMD.

Fully sharded design (v2):
- Core r owns nodes [r*5120, (r+1)*5120); ALL dense compute is sharded 8x.
  BN batch stats are per-shard partial sums + a small AllReduce; the 960
  padding nodes (40000 -> 40960) are corrected analytically by tracking the
  (identical) pad-node column p through every transform.
- Neighbor aggregation uses matmul commutation:
      segment_sum((x@Wnb)[row], col) = segment_sum(x[row], col) @ Wnb
  so the gather table is x itself (node-major, AllGather'd once per layer);
  no per-layer h-table pass.
- Gather: dma_gather of 256B rows (int16 half-relative indices), scatter-add
  via PE one-hot matmuls (fp8 one-hots SBUF-resident, loaded once).
- Edge branch ea = segment_sum(edge_attr@Wedge + bedge, row) factors into
  rank-2 outer products of (s, c_out); its BN stats come in closed form from
  5 scalar moments of (s, c_out); s = segment_sum(edge_attr, row) is computed
  once on device by a row-sorted one-hot matmul pass and stays core-local.
- y1 = relu(bn_n(xn) + bn_a(aggr) + bn_e(ea)) is computed as a single PSUM
  accumulation: x@(Wn diag(sc_n)) + agx@(Wnb diag(sc_a)) + lhsT3-outer, with
  the scale-folded weights built on device via diag matmuls.
"""
import numpy as np
import ml_dtypes

F = 128
L = 3
EPS = 1e-5
NREAL = 40000
NE = 640000
NCORES = 8
NP_ = 40960
SHARD = NP_ // NCORES       # 5120
HALF = NP_ // 2             # 20480
NBLK = SHARD // F           # 40 blocks per core
CHUNK = 512
NCH = SHARD // CHUNK        # 10 dense chunks per core
NPAD = NP_ - NREAL          # 960

F16 = np.float16
FP8 = ml_dtypes.float8_e4m3


def _ceil(a, b):
    return -(-a // b)


def _wrap_idx16(vals):
    """int16 gather-index layout: value j at [j%16, j//16], tiled to 128 parts."""
    n = vals.shape[0]
    a = vals.reshape(n // 16, 16).T.astype(np.int16)   # [16, n/16]
    return np.tile(a, (8, 1))                          # [128, n/16]


def _prep(node_attr, edge_index, edge_attr):
    """Host-side index preprocessing -> per-core arrays + metadata."""
    row = edge_index[0].astype(np.int64)
    col = edge_index[1].astype(np.int64)
    ea = edge_attr[:, 0].astype(np.float32)

    shard = col // SHARD
    half = row // HALF
    blk = (col % SHARD) // F
    tloc = col % F

    # --- col pass (neighbor aggregation of raw x) ---
    cnt = np.zeros((NCORES, 2, NBLK), np.int64)
    np.add.at(cnt, (shard, half, blk), 1)
    sseg = _ceil(max(int(cnt.max()), 1), F) * F
    cpb = sseg // F                   # chunks per (half, block)
    nch_h = NBLK * cpb                # chunks per half
    nstream_h = NBLK * sseg           # slots per half
    nch = 2 * nch_h
    order = np.lexsort((blk, half, shard))
    so_shard, so_half, so_blk = shard[order], half[order], blk[order]
    grp = ((so_shard * 2 + so_half) * NBLK + so_blk)
    grp_start = np.zeros(NCORES * 2 * NBLK + 1, np.int64)
    np.add.at(grp_start, grp + 1, 1)
    grp_start = np.cumsum(grp_start)
    within = np.arange(NE) - grp_start[grp]
    slot = (so_half * NBLK + so_blk) * sseg + within   # slot in core stream

    gsrc = np.broadcast_to(
        (np.arange(2 * nstream_h) % HALF).astype(np.int16),
        (NCORES, 2 * nstream_h)).copy()
    gsrc[so_shard, slot] = (row[order] - so_half * HALF).astype(np.int16)
    gidx = np.stack([_wrap_idx16(gsrc[r]) for r in range(NCORES)])  # [8,128,S/16]

    # one-hot flat for SBUF residency: [core, 128 part(slot%128), nch*F]
    ohc = np.zeros((NCORES, 128, nch * F), FP8)
    ohc[so_shard, slot % F, (slot // F) * F + tloc[order]] = 1.0

    # --- row pass (s = segment_sum(edge_attr, row)) ---
    rshard = row // SHARD
    rblk = (row % SHARD) // F
    rloc = row % F
    rcnt = np.zeros((NCORES, NBLK), np.int64)
    np.add.at(rcnt, (rshard, rblk), 1)
    rseg = _ceil(max(int(rcnt.max()), 1), F) * F
    cpb2 = rseg // F
    nch2 = NBLK * cpb2
    rorder = np.lexsort((rblk, rshard))
    ro_shard, ro_blk = rshard[rorder], rblk[rorder]
    rgrp = ro_shard * NBLK + ro_blk
    rgs = np.zeros(NCORES * NBLK + 1, np.int64)
    np.add.at(rgs, rgrp + 1, 1)
    rgs = np.cumsum(rgs)
    rwithin = np.arange(NE) - rgs[rgrp]
    rslot = ro_blk * rseg + rwithin

    eav = np.zeros((NCORES, NBLK * rseg), np.float32)
    eav[ro_shard, rslot] = ea[rorder]
    ohr = np.zeros((NCORES, nch2 // 8, 128, 8, F), FP8)
    ohr[ro_shard, (rslot // F) // 8, rslot % F, (rslot // F) % 8, rloc[rorder]] = 1.0
    eav_t = np.ascontiguousarray(
        eav.reshape(NCORES, nch2, F).transpose(0, 2, 1)).astype(F16)

    # degree counts (pure edge_index metadata), per-core shard slices
    c_out = np.bincount(row, minlength=NP_).astype(np.float32)
    c_in = np.bincount(col, minlength=NP_).astype(np.float32)
    cip = np.stack([c_in, c_out]).reshape(2, NCORES, SHARD).transpose(1, 0, 2)
    cip = np.ascontiguousarray(cip).astype(F16)   # [8, 2=(c_in,c_out), SHARD]

    naT = np.zeros((2, NP_), np.float32)
    naT[:, :NREAL] = node_attr.T
    naT = np.ascontiguousarray(
        naT.reshape(2, NCORES, SHARD).transpose(1, 0, 2)).astype(F16)  # [8,2,SHARD]

    return dict(sseg=sseg, cpb=cpb, nch=nch, nch_h=nch_h, nstream_h=nstream_h,
                rseg=rseg, cpb2=cpb2, nch2=nch2,
                gidx=gidx, ohc=ohc, ohr=ohr, eav_t=eav_t,
                cip=cip, naT=naT)


def _build(meta):
    """Build the Bass program."""
    import os
    DBG_NO_SCATTER = bool(int(os.environ.get("K_NO_SCATTER", "0")))
    DBG_NO_CC = bool(int(os.environ.get("K_NO_CC", "0")))
    import concourse.bass as bass
    import concourse.tile as tile
    from concourse import bacc, mybir

    sseg, cpb, nch_h = meta["sseg"], meta["cpb"], meta["nch_h"]
    nstream_h = meta["nstream_h"]
    nch = meta["nch"]
    cpb2, nch2 = meta["cpb2"], meta["nch2"]
    GCALL = 1024                      # 64 descriptors/engine = one packet
    NCALLS_H = nstream_h // GCALL     # = 5*cpb (45 for cpb=9)
    KPC = GCALL // F                  # chunks per gather call = 8
    dt = mybir.dt
    AX = mybir.AxisListType.X
    OP = mybir.AluOpType
    ACTF = mybir.ActivationFunctionType

    nc = bacc.Bacc("TRN2", target_bir_lowering=False, debug=False,
                   num_devices=NCORES, num_swdge_queues=4)

    def din(name, shape, d):
        return nc.dram_tensor(name, shape, d, kind="ExternalInput")

    naT = din("naT", [2, SHARD], dt.float16)
    cip = din("cip", [2, SHARD], dt.float16)
    gidx = din("gidx", [128, 2 * nstream_h // 16], dt.int16)
    ohc = din("ohc", [128, nch * F], dt.float8e4)
    ohr = din("ohr", [nch2 // 8, 128, 8, F], dt.float8e4)
    eav = din("eav", [128, nch2], dt.float16)
    W0 = din("W0", [2, F], dt.float16)
    Wn = [din(f"Wn{i}", [F, F], dt.float16) for i in range(L)]
    WnT = [din(f"WnT{i}", [F, F], dt.float16) for i in range(L)]
    Wb = [din(f"Wb{i}", [F, F], dt.float16) for i in range(L)]
    WbT = [din(f"WbT{i}", [F, F], dt.float16) for i in range(L)]
    W1 = [din(f"W1{i}", [F, F], dt.float16) for i in range(L)]
    W2 = [din(f"W2{i}", [F, F], dt.float16) for i in range(L)]
    WecA = [din(f"WecA{i}", [1, F], dt.float32) for i in range(L)]    # w_e row
    WecB = [din(f"WecB{i}", [1, F], dt.float32) for i in range(L)]    # b_e row
    WecC = [din(f"WecC{i}", [F, 2], dt.float32) for i in range(L)]    # cols
    BnbR = [din(f"BnbR{i}", [1, F], dt.float16) for i in range(L)]    # bnb row
    I16 = din("I16", [F, F], dt.float16)
    I32 = din("I32", [F, F], dt.float32)
    gcol = {}
    for nm in ("g0", "bt0"):
        gcol[nm] = din(nm, [F, 1], dt.float32)
    for i in range(L):
        for nm in ("gn", "btn", "ge", "bte", "gnb", "btnb",
                   "gm1", "btm1", "gm2", "btm2"):
            gcol[f"{nm}{i}"] = din(f"{nm}{i}", [F, 1], dt.float32)

    out = nc.dram_tensor("out", [F, SHARD], dt.float32, kind="ExternalOutput")

    with tile.TileContext(nc) as tc:
        import contextlib
        ctx = contextlib.ExitStack()
        with ctx:
            sb = ctx.enter_context(tc.tile_pool(name="sb", bufs=1))
            wpool = ctx.enter_context(tc.tile_pool(name="wp", bufs=1))
            tp = ctx.enter_context(tc.tile_pool(name="tp", bufs=2))
            gp = ctx.enter_context(tc.tile_pool(name="gp", bufs=2))
            ohrp = ctx.enter_context(tc.tile_pool(name="ohrp", bufs=2))
            stp = ctx.enter_context(tc.tile_pool(name="stp", bufs=1))
            afp = ctx.enter_context(tc.tile_pool(name="afp", bufs=4))
            ps_a = ctx.enter_context(tc.tile_pool(name="psa", bufs=2, space="PSUM"))
            ps_b = ctx.enter_context(tc.tile_pool(name="psb", bufs=2, space="PSUM"))
            ps_sc = ctx.enter_context(tc.tile_pool(name="pssc", bufs=2, space="PSUM"))
            ps_sm = ctx.enter_context(tc.tile_pool(name="pssm", bufs=1, space="PSUM"))
            ps_tp = ctx.enter_context(tc.tile_pool(name="pstp", bufs=1, space="PSUM"))
            dram = ctx.enter_context(tc.tile_pool(name="dram", bufs=1, space="DRAM"))

            gsems = [nc.alloc_semaphore(f"gsem{q}") for q in range(4)]
            # ---- persistent SBUF ----
            xbuf = sb.tile([F, SHARD], dt.float16)
            agx = sb.tile([F, SHARD], dt.float16)
            scv = sb.tile([3, SHARD], dt.float16)      # rows: c_in, s, c_out
            nc.sync.dma_start(scv[0:1, :], cip.ap()[0:1, :])
            nc.sync.dma_start(scv[2:3, :], cip.ap()[1:2, :])
            s_row = sb.tile([1, SHARD], dt.float16)
            cout_row = sb.tile([1, SHARD], dt.float16)
            nc.sync.dma_start(cout_row[:], cip.ap()[1:2, :])
            gidx_sb = sb.tile([128, 2 * nstream_h // 16], dt.int16)
            nc.sync.dma_start(gidx_sb[:], gidx.ap())
            eav_sb = sb.tile([128, nch2], dt.float16)
            nc.sync.dma_start(eav_sb[:], eav.ap())
            trash = sb.tile([F, CHUNK], dt.float32)
            psmall = ps_sm.tile([F, F], dt.float32, tag="small")

            def wload(t_, tag):
                w = wpool.tile(list(t_.shape), t_.dtype, tag=tag)
                nc.sync.dma_start(w[:], t_.ap())
                return w

            W0_sb = wload(W0, "w0")
            I16_sb = wload(I16, "i16")
            I32_sb = wload(I32, "i32")
            Wn_sb = [wload(Wn[i], f"wn{i}") for i in range(L)]
            WnT_sb = [wload(WnT[i], f"wnt{i}") for i in range(L)]
            Wb_sb = [wload(Wb[i], f"wb{i}") for i in range(L)]
            WbT_sb = [wload(WbT[i], f"wbt{i}") for i in range(L)]
            W1_sb = [wload(W1[i], f"w1{i}") for i in range(L)]
            W2_sb = [wload(W2[i], f"w2{i}") for i in range(L)]
            WecA_sb = [wload(WecA[i], f"weca{i}") for i in range(L)]
            WecB_sb = [wload(WecB[i], f"wecb{i}") for i in range(L)]
            WecC_sb = [wload(WecC[i], f"wecc{i}") for i in range(L)]
            BnbR_sb = [wload(BnbR[i], f"bnbr{i}") for i in range(L)]
            gc_sb = {nm: wload(t_, f"p{nm}") for nm, t_ in gcol.items()}

            # ---- DRAM scratch ----
            ag_ins = [dram.tile([SHARD, F], dt.float16, tag=f"agi{i}",
                                name=f"agi{i}") for i in range(L)]
            agos = [dram.tile([NP_, F], dt.float16, addr_space="Shared",
                              tag=f"ago{i}", name=f"ago{i}") for i in range(L)]
            htabs = [dram.tile([HALF, F], dt.float16, tag=f"htab{h}",
                               name=f"htab{h}") for h in range(2)]
            ar_ins, ar_outs = [], []

            def make_ar(ncols, tag):
                i_ = dram.tile([F, ncols], dt.float32, tag=f"ari{tag}",
                               name=f"ari{tag}")
                o_ = dram.tile([F, ncols], dt.float32, addr_space="Shared",
                               tag=f"aro{tag}", name=f"aro{tag}")
                return i_, o_

            # ---------- helpers ----------
            def fire_ar(buf, ncols, tag):
                if DBG_NO_CC:
                    rb = stp.tile([F, ncols], dt.float32, tag=f"rb{tag}")
                    nc.vector.tensor_scalar_mul(rb[:], buf[:, :ncols],
                                                float(NCORES))
                    return rb
                ari, aro = make_ar(ncols, tag)
                nc.gpsimd.dma_start(ari[:], buf[:, :ncols])
                nc.gpsimd.collective_compute(
                    "AllReduce", OP.add, replica_groups=[list(range(NCORES))],
                    ins=[ari.opt()], outs=[aro.opt()])
                rb = stp.tile([F, ncols], dt.float32, tag=f"rb{tag}")
                nc.sync.dma_start(rb[:], aro[:])
                return rb

            def fin_mv(g, bt, mean, var):
                """(scale, shift) from mean/var columns [F,1]."""
                v2 = afp.tile([F, 1], dt.float32)
                nc.vector.tensor_scalar_add(v2[:], var[:], EPS)
                lnv = afp.tile([F, 1], dt.float32)
                nc.scalar.activation(lnv[:], v2[:], ACTF.Ln)
                isig = afp.tile([F, 1], dt.float32)
                nc.scalar.activation(isig[:], lnv[:], ACTF.Exp, scale=-0.5)
                scale = afp.tile([F, 1], dt.float32)
                nc.vector.tensor_mul(scale[:], g[:], isig[:])
                nscale = afp.tile([F, 1], dt.float32)
                nc.vector.tensor_scalar_mul(nscale[:], scale[:], -1.0)
                shift = afp.tile([F, 1], dt.float32)
                nc.vector.scalar_tensor_tensor(
                    out=shift[:], in0=mean[:], scalar=nscale[:], in1=bt[:],
                    op0=OP.mult, op1=OP.add)
                return scale, shift

            def fin_sums(g, bt, ssum, ssq, corr=None):
                """(scale, shift) from global sum/sumsq [F,1]; corr = pad col."""
                if corr is not None:
                    c2 = afp.tile([F, 1], dt.float32)
                    nc.scalar.activation(c2[:], corr[:], ACTF.Square)
                    nc.vector.scalar_tensor_tensor(
                        out=ssum[:], in0=corr[:], scalar=-float(NPAD), in1=ssum[:],
                        op0=OP.mult, op1=OP.add)
                    nc.vector.scalar_tensor_tensor(
                        out=ssq[:], in0=c2[:], scalar=-float(NPAD), in1=ssq[:],
                        op0=OP.mult, op1=OP.add)
                mean = afp.tile([F, 1], dt.float32)
                nc.vector.tensor_scalar_mul(mean[:], ssum[:], 1.0 / NREAL)
                m2t = afp.tile([F, 1], dt.float32)
                nc.scalar.activation(m2t[:], mean[:], ACTF.Square)
                var = afp.tile([F, 1], dt.float32)
                nc.vector.scalar_tensor_tensor(
                    out=var[:], in0=ssq[:], scalar=1.0 / NREAL, in1=m2t[:],
                    op0=OP.mult, op1=OP.subtract)
                sc, sh = fin_mv(g, bt, mean, var)
                return sc, sh, mean

            def stat_chunk(ps, c, ss, sq):
                nc.vector.tensor_reduce(ss[:, c:c + 1], ps[:, :], AX, OP.add)
                nc.scalar.activation(trash[:], ps[:, :], ACTF.Square,
                                     accum_out=sq[:, c:c + 1])

            def colred(sl):
                r = afp.tile([F, 1], dt.float32)
                nc.vector.tensor_reduce(r[:], sl[:], AX, OP.add)
                return r

            def col16(col):
                t = afp.tile([F, 1], dt.float16)
                nc.vector.tensor_copy(t[:], col[:])
                return t

            def pad_mm(w_sb, col_f16):
                """[F,1] = w.T @ col via PE; returns fp32 sbuf col."""
                nc.tensor.matmul(psmall[:, 0:1], lhsT=w_sb[:], rhs=col_f16[:],
                                 start=True, stop=True)
                o = afp.tile([F, 1], dt.float32)
                nc.vector.tensor_copy(o[:], psmall[:, 0:1])
                return o

            def scaled_w(wT_sb, sc):
                """W*diag(sc) as fp16 SBUF tile, via diag matmul."""
                dg = tp.tile([F, F], dt.float16, tag="diag")
                nc.vector.tensor_scalar_mul(dg[:], I32_sb[:], sc[:])
                nc.tensor.matmul(psmall[:], lhsT=wT_sb[:], rhs=dg[:],
                                 start=True, stop=True)
                o = stp.tile([F, F], dt.float16, tag=f"wsc{scaled_w.n}")
                scaled_w.n += 1
                nc.scalar.activation(o[:], psmall[:], ACTF.Copy)
                return o
            scaled_w.n = 0

            # ---------- P0: bn0 ----------
            ss0 = stp.tile([F, NCH], dt.float32, tag="ss0")
            sq0 = stp.tile([F, NCH], dt.float32, tag="sq0")
            for c in range(NCH):
                sl = slice(c * CHUNK, (c + 1) * CHUNK)
                nat = tp.tile([2, CHUNK], dt.float16, tag="nat")
                nc.sync.dma_start(nat[:], naT.ap()[:, sl])
                ps = ps_a.tile([F, CHUNK], dt.float32, tag="mm")
                nc.tensor.matmul(ps[:], lhsT=W0_sb[:], rhs=nat[:],
                                 start=True, stop=True)
                stat_chunk(ps, c, ss0, sq0)
                nc.scalar.activation(xbuf[:, sl], ps[:], ACTF.Copy)
            arbuf0 = stp.tile([F, 2], dt.float32, tag="arb0")
            nc.vector.tensor_copy(arbuf0[:, 0:1], colred(ss0)[:])
            nc.vector.tensor_copy(arbuf0[:, 1:2], colred(sq0)[:])
            rb0 = fire_ar(arbuf0, 2, "ar0")
            g_ss = afp.tile([F, 1], dt.float32)
            nc.vector.tensor_copy(g_ss[:], rb0[:, 0:1])
            g_sq = afp.tile([F, 1], dt.float32)
            nc.vector.tensor_copy(g_sq[:], rb0[:, 1:2])
            sc0, sh0, _ = fin_sums(gc_sb["g0"], gc_sb["bt0"], g_ss, g_sq)
            for c in range(NCH):
                sl = slice(c * CHUNK, (c + 1) * CHUNK)
                nc.scalar.activation(xbuf[:, sl], xbuf[:, sl], ACTF.Relu,
                                     bias=sh0[:], scale=sc0[:])
            p_col = afp.tile([F, 1], dt.float32)   # pad-node column (fp32)
            nc.scalar.activation(p_col[:], sh0[:], ACTF.Relu)

            # ---------- transpose x0 + AG#0 ----------
            def ship_table(ag_in):
                for b in range(NBLK):
                    bs = slice(b * F, (b + 1) * F)
                    pt = ps_tp.tile([F, F], dt.float16, tag="tpt")
                    nc.tensor.transpose(pt[:], xbuf[:, bs], I16_sb[:])
                    tb = tp.tile([F, F], dt.float16, tag="tb")
                    nc.vector.tensor_copy(tb[:], pt[:])
                    nc.sync.dma_start(ag_in[bs, :], tb[:])

            ship_table(ag_ins[0])
            if not DBG_NO_CC:
                nc.gpsimd.collective_compute(
                    "AllGather", OP.bypass, replica_groups=[list(range(NCORES))],
                    ins=[ag_ins[0].opt()], outs=[agos[0].opt()])

            # ---------- s-pass ----------
            for b in range(NBLK):
                pss = psmall[0:1, :]
                for k in range(cpb2):
                    ci = b * cpb2 + k
                    if ci % 8 == 0:
                        ohrt = ohrp.tile([128, 8, F], dt.float8e4, tag="ohr")
                        nc.sync.dma_start(ohrt[:], ohr.ap()[ci // 8])
                    nc.tensor.matmul(pss, lhsT=eav_sb[:, ci:ci + 1],
                                     rhs=ohrt[:, ci % 8, :],
                                     start=(k == 0), stop=(k == cpb2 - 1))
                nc.vector.tensor_copy(s_row[:, b * F:(b + 1) * F], pss)

            s_bounce = dram.tile([1, SHARD], dt.float16, tag="sbnc", name="sbnc")
            nc.sync.dma_start(s_bounce[:], s_row[:])
            nc.sync.dma_start(scv[1:2, :], s_bounce[:])
            # moments of (s, c_out) over local shard -> [1,5] partials
            momp = stp.tile([1, 8], dt.float32, tag="momp")
            nc.vector.tensor_reduce(momp[:, 0:1], s_row[:], AX, OP.add)
            nc.vector.tensor_reduce(momp[:, 1:2], cout_row[:], AX, OP.add)
            t_mom = stp.tile([1, SHARD], dt.float16, tag="tmom")
            nc.vector.tensor_mul(t_mom[:], s_row[:], cout_row[:])
            nc.vector.tensor_reduce(momp[:, 2:3], t_mom[:], AX, OP.add)
            nc.scalar.activation(t_mom[:], s_row[:], ACTF.Square,
                                 accum_out=momp[:, 3:4])
            nc.scalar.activation(t_mom[:], cout_row[:], ACTF.Square,
                                 accum_out=momp[:, 4:5])
            ones_row = stp.tile([1, F], dt.float32, tag="ones")
            nc.vector.memset(ones_row[:], 1.0)
            mom_bc = None   # [F,5] fp32 broadcast moments (set at layer 0)

            # ---------- layers ----------
            sc2 = sh2 = None
            for i in range(L):
                # --- P1: xn stats (overlaps scatter) ---
                ssn = stp.tile([F, NCH], dt.float32, tag=f"ssn{i}")
                sqn = stp.tile([F, NCH], dt.float32, tag=f"sqn{i}")
                for c in range(NCH):
                    sl = slice(c * CHUNK, (c + 1) * CHUNK)
                    ps = ps_a.tile([F, CHUNK], dt.float32, tag="mm")
                    nc.tensor.matmul(ps[:], lhsT=Wn_sb[i][:], rhs=xbuf[:, sl],
                                     start=True, stop=True)
                    stat_chunk(ps, c, ssn, sqn)
                p16 = col16(p_col)
                q_n = pad_mm(Wn_sb[i], p16)

                # --- scatter phase ---
                if DBG_NO_SCATTER:
                    nc.vector.memset(agx[:], 0.0)
                for h in range(2 if not DBG_NO_SCATTER else 0):
                    for call in range(NCALLS_H):
                        gt = gp.tile([128, KPC, F], dt.float16, tag="g")
                        j0 = h * nstream_h + call * GCALL
                        nc.gpsimd.dma_gather(
                            out_ap=gt[:],
                            in_ap=agos[i][h * HALF:(h + 1) * HALF, :],
                            idxs_ap=gidx_sb[:, j0 // 16:(j0 + GCALL) // 16],
                            num_idxs=GCALL, num_idxs_reg=GCALL, elem_size=F,
                            prepare_only=True, sem=gsems[call % 4],
                            queue_num=call % 4)
                        nc.gpsimd.trigger_dma(count=None, queue_num=call % 4)
                        for k8 in range(KPC):
                            ci = call * KPC + k8          # chunk within half
                            b = ci // cpb
                            k = ci % cpb
                            if k == 0:
                                psb = ps_sc.tile([F, F], dt.float32, tag="sc")
                            nc.tensor.matmul(
                                psb[:], lhsT=gt[:, k8, :],
                                rhs=ohc_sb[:, (h * nch_h + ci) * F:
                                           (h * nch_h + ci + 1) * F],
                                start=(k == 0), stop=(k == cpb - 1))
                            if k == cpb - 1:
                                dst = agx[:, b * F:(b + 1) * F]
                                if h == 0:
                                    nc.scalar.activation(dst, psb[:], ACTF.Copy)
                                else:
                                    nc.vector.scalar_tensor_tensor(
                                        out=dst, in0=psb[:], scalar=1.0,
                                        in1=dst, op0=OP.mult, op1=OP.add)

                # --- P2: aggr stats ---
                ssa = stp.tile([F, NCH], dt.float32, tag=f"ssa{i}")
                sqa = stp.tile([F, NCH], dt.float32, tag=f"sqa{i}")
                for c in range(NCH):
                    sl = slice(c * CHUNK, (c + 1) * CHUNK)
                    ps = ps_a.tile([F, CHUNK], dt.float32, tag="mm")
                    nc.tensor.matmul(ps[:], lhsT=Wb_sb[i][:], rhs=agx[:, sl],
                                     start=True, stop=False)
                    nc.tensor.matmul(ps[:], lhsT=BnbR_sb[i][:], rhs=scv[0:1, sl],
                                     start=False, stop=True)
                    stat_chunk(ps, c, ssa, sqa)
                if i == 0:
                    arb = stp.tile([F, 9], dt.float32, tag="arb1a0")
                    nc.vector.memset(arb[:], 0.0)
                    nc.vector.tensor_copy(arb[0:1, 4:9], momp[:, 0:5])
                else:
                    arb = stp.tile([F, 4], dt.float32, tag=f"arb1a{i}")
                nc.vector.tensor_copy(arb[:, 0:1], colred(ssn)[:])
                nc.vector.tensor_copy(arb[:, 1:2], colred(sqn)[:])
                nc.vector.tensor_copy(arb[:, 2:3], colred(ssa)[:])
                nc.vector.tensor_copy(arb[:, 3:4], colred(sqa)[:])
                rb1a = fire_ar(arb, 9 if i == 0 else 4, f"ar1a{i}")
                rb1b = rb1a

                # --- finalize n / a / e ---
                if i == 0:
                    nc.tensor.matmul(psmall[:, 0:5], lhsT=ones_row[:],
                                     rhs=rb1a[0:1, 4:9], start=True, stop=True)
                    mom_bc = stp.tile([F, 5], dt.float32, tag="mombc")
                    nc.vector.tensor_scalar_mul(mom_bc[:], psmall[:, 0:5], 1.0 / NREAL)
                n_ss = afp.tile([F, 1], dt.float32)
                nc.vector.tensor_copy(n_ss[:], rb1a[:, 0:1])
                n_sq = afp.tile([F, 1], dt.float32)
                nc.vector.tensor_copy(n_sq[:], rb1a[:, 1:2])
                sc_n, sh_n, _ = fin_sums(gc_sb[f"gn{i}"], gc_sb[f"btn{i}"],
                                         n_ss, n_sq, corr=q_n)
                a_ss = afp.tile([F, 1], dt.float32)
                nc.vector.tensor_copy(a_ss[:], rb1b[:, 2:3])
                a_sq = afp.tile([F, 1], dt.float32)
                nc.vector.tensor_copy(a_sq[:], rb1b[:, 3:4])
                sc_a, sh_a, _ = fin_sums(gc_sb[f"gnb{i}"], gc_sb[f"btnb{i}"],
                                         a_ss, a_sq)
                # analytic ea stats: mean = w*mu_s + b*mu_c
                # E2 = w^2*Mss + 2wb*Msc + b^2*Mcc    (mom cols: mu_s,mu_c,Msc,Mss,Mcc)
                wcol, bcol = WecC_sb[i][:, 0:1], WecC_sb[i][:, 1:2]
                me = afp.tile([F, 1], dt.float32)
                nc.vector.tensor_mul(me[:], wcol, mom_bc[:, 0:1])
                nc.vector.scalar_tensor_tensor(
                    out=me[:], in0=bcol, scalar=mom_bc[:, 1:2], in1=me[:],
                    op0=OP.mult, op1=OP.add)
                w2 = afp.tile([F, 1], dt.float32)
                nc.scalar.activation(w2[:], wcol, ACTF.Square)
                b2 = afp.tile([F, 1], dt.float32)
                nc.scalar.activation(b2[:], bcol, ACTF.Square)
                wb2 = afp.tile([F, 1], dt.float32)
                nc.vector.tensor_mul(wb2[:], wcol, bcol)
                e2 = afp.tile([F, 1], dt.float32)
                nc.vector.tensor_mul(e2[:], w2[:], mom_bc[:, 3:4])
                nc.vector.scalar_tensor_tensor(
                    out=e2[:], in0=wb2[:], scalar=mom_bc[:, 2:3], in1=e2[:],
                    op0=OP.mult, op1=OP.add)
                nc.vector.scalar_tensor_tensor(
                    out=e2[:], in0=wb2[:], scalar=mom_bc[:, 2:3], in1=e2[:],
                    op0=OP.mult, op1=OP.add)
                nc.vector.scalar_tensor_tensor(
                    out=e2[:], in0=b2[:], scalar=mom_bc[:, 4:5], in1=e2[:],
                    op0=OP.mult, op1=OP.add)
                me2 = afp.tile([F, 1], dt.float32)
                nc.scalar.activation(me2[:], me[:], ACTF.Square)
                ve = afp.tile([F, 1], dt.float32)
                nc.vector.tensor_sub(ve[:], e2[:], me2[:])
                sc_e, sh_e = fin_mv(gc_sb[f"ge{i}"], gc_sb[f"bte{i}"], me, ve)
                # combined shift; scale-folded weights; outer lhsT3
                shsum = afp.tile([F, 1], dt.float32)
                nc.vector.tensor_add(shsum[:], sh_n[:], sh_e[:])
                nc.vector.tensor_add(shsum[:], shsum[:], sh_a[:])
                Wn_sc = scaled_w(WnT_sb[i], sc_n)
                Wb_sc = scaled_w(WbT_sb[i], sc_a)
                # lhsT3 rows pair with scv rows (c_in, s, c_out):
                # (sc_a*bnb, sc_e*w_e, sc_e*b_e); assembled via SBUF DMAs
                nc.tensor.transpose(psmall[0:1, :], sc_e[:], I32_sb[:])
                sce_row = stp.tile([1, F], dt.float32, tag=f"scer{i}")
                nc.vector.tensor_copy(sce_row[:], psmall[0:1, :])
                nc.tensor.transpose(psmall[0:1, :], sc_a[:], I32_sb[:])
                sca_row = stp.tile([1, F], dt.float32, tag=f"scar{i}")
                nc.vector.tensor_copy(sca_row[:], psmall[0:1, :])
                l3r = [stp.tile([1, F], dt.float16, tag=f"l3r{k}_{i}",
                                name=f"l3r{k}_{i}") for k in range(3)]
                nc.vector.tensor_mul(l3r[0][:], BnbR_sb[i][:], sca_row[:])
                nc.vector.tensor_mul(l3r[1][:], WecA_sb[i][:], sce_row[:])
                nc.vector.tensor_mul(l3r[2][:], WecB_sb[i][:], sce_row[:])

                # --- P3: y1 + m1 ---
                ss1 = stp.tile([F, NCH], dt.float32, tag=f"ss1{i}")
                sq1 = stp.tile([F, NCH], dt.float32, tag=f"sq1{i}")
                for c in range(NCH):
                    sl = slice(c * CHUNK, (c + 1) * CHUNK)
                    ps = ps_a.tile([F, CHUNK], dt.float32, tag="mm")
                    nc.tensor.matmul(ps[:], lhsT=Wn_sc[:], rhs=xbuf[:, sl],
                                     start=True, stop=False)
                    nc.tensor.matmul(ps[:], lhsT=Wb_sc[:], rhs=agx[:, sl],
                                     start=False, stop=False)
                    nc.tensor.matmul(ps[:], lhsT=l3r[0][:], rhs=scv[0:1, sl],
                                     start=False, stop=False)
                    nc.tensor.matmul(ps[:], lhsT=l3r[1][:], rhs=s_row[:, sl],
                                     start=False, stop=False)
                    nc.tensor.matmul(ps[:], lhsT=l3r[2][:], rhs=cout_row[:, sl],
                                     start=False, stop=True)
                    y1 = tp.tile([F, CHUNK], dt.float16, tag="y1")
                    nc.scalar.activation(y1[:], ps[:], ACTF.Relu, bias=shsum[:])
                    pm = ps_b.tile([F, CHUNK], dt.float32, tag="pm")
                    nc.tensor.matmul(pm[:], lhsT=W1_sb[i][:], rhs=y1[:],
                                     start=True, stop=True)
                    stat_chunk(pm, c, ss1, sq1)
                    nc.scalar.activation(xbuf[:, sl], pm[:], ACTF.Copy)
                r1 = afp.tile([F, 1], dt.float32)
                nc.scalar.activation(r1[:], q_n[:], ACTF.Relu,
                                     bias=shsum[:], scale=sc_n[:])
                m1p = pad_mm(W1_sb[i], col16(r1))
                arb2 = stp.tile([F, 2], dt.float32, tag=f"arb2{i}")
                nc.vector.tensor_copy(arb2[:, 0:1], colred(ss1)[:])
                nc.vector.tensor_copy(arb2[:, 1:2], colred(sq1)[:])
                rb2 = fire_ar(arb2, 2, f"ar2{i}")
                m_ss = afp.tile([F, 1], dt.float32)
                nc.vector.tensor_copy(m_ss[:], rb2[:, 0:1])
                m_sq = afp.tile([F, 1], dt.float32)
                nc.vector.tensor_copy(m_sq[:], rb2[:, 1:2])
                sc1, sh1, _ = fin_sums(gc_sb[f"gm1{i}"], gc_sb[f"btm1{i}"],
                                       m_ss, m_sq, corr=m1p)

                # --- P4: y2 + m2 ---
                ss2 = stp.tile([F, NCH], dt.float32, tag=f"ss2{i}")
                sq2 = stp.tile([F, NCH], dt.float32, tag=f"sq2{i}")
                for c in range(NCH):
                    sl = slice(c * CHUNK, (c + 1) * CHUNK)
                    y2 = tp.tile([F, CHUNK], dt.float16, tag="y2")
                    nc.scalar.activation(y2[:], xbuf[:, sl], ACTF.Relu,
                                         bias=sh1[:], scale=sc1[:])
                    pm = ps_b.tile([F, CHUNK], dt.float32, tag="pm")
                    nc.tensor.matmul(pm[:], lhsT=W2_sb[i][:], rhs=y2[:],
                                     start=True, stop=True)
                    stat_chunk(pm, c, ss2, sq2)
                    nc.scalar.activation(xbuf[:, sl], pm[:], ACTF.Copy)
                y2p = afp.tile([F, 1], dt.float32)
                nc.scalar.activation(y2p[:], m1p[:], ACTF.Relu,
                                     bias=sh1[:], scale=sc1[:])
                m2p = pad_mm(W2_sb[i], col16(y2p))
                arb3 = stp.tile([F, 2], dt.float32, tag=f"arb3{i}")
                nc.vector.tensor_copy(arb3[:, 0:1], colred(ss2)[:])
                nc.vector.tensor_copy(arb3[:, 1:2], colred(sq2)[:])
                rb3 = fire_ar(arb3, 2, f"ar3{i}")
                m2ss = afp.tile([F, 1], dt.float32)
                nc.vector.tensor_copy(m2ss[:], rb3[:, 0:1])
                m2sq = afp.tile([F, 1], dt.float32)
                nc.vector.tensor_copy(m2sq[:], rb3[:, 1:2])
                sc2, sh2, _ = fin_sums(gc_sb[f"gm2{i}"], gc_sb[f"btm2{i}"],
                                       m2ss, m2sq, corr=m2p)

                # --- P5: x_next (or output) ---
                if i < L - 1:
                    for c in range(NCH):
                        sl = slice(c * CHUNK, (c + 1) * CHUNK)
                        nc.scalar.activation(xbuf[:, sl], xbuf[:, sl], ACTF.Relu,
                                             bias=sh2[:], scale=sc2[:])
                    pnew = afp.tile([F, 1], dt.float32)
                    nc.scalar.activation(pnew[:], m2p[:], ACTF.Relu,
                                         bias=sh2[:], scale=sc2[:])
                    p_col = pnew
                    ship_table(ag_ins[i + 1])
                    if not DBG_NO_CC:
                        nc.gpsimd.collective_compute(
                            "AllGather", OP.bypass,
                            replica_groups=[list(range(NCORES))],
                            ins=[ag_ins[i + 1].opt()], outs=[agos[i + 1].opt()])
                else:
                    for c in range(NCH):
                        sl = slice(c * CHUNK, (c + 1) * CHUNK)
                        of = tp.tile([F, CHUNK], dt.float32, tag="of")
                        nc.scalar.activation(of[:], xbuf[:, sl], ACTF.Relu,
                                             bias=sh2[:], scale=sc2[:])
                        nc.sync.dma_start(out.ap()[:, sl], of[:])

    nc.compile()
    return nc


def kernel(**inputs):
    import sys
    for p in ("/opt/trn_rl_repo",):
        if p not in sys.path:
            sys.path.insert(0, p)
    from concourse import bass_utils

    meta = _prep(inputs["node_attr"], inputs["edge_index"], inputs["edge_attr"])
    nc = _build(meta)

    def col(v):
        return np.ascontiguousarray(v.astype(np.float32).reshape(F, 1))

    base = dict(
        W0=inputs["W0"].astype(F16),
        I16=np.eye(F, dtype=F16),
        I32=np.eye(F, dtype=np.float32),
        g0=col(inputs["g0"]), bt0=col(inputs["bt0"]),
    )
    for i in range(L):
        base[f"Wn{i}"] = inputs["Wnode"][i].astype(F16)
        base[f"WnT{i}"] = np.ascontiguousarray(inputs["Wnode"][i].T).astype(F16)
        base[f"Wb{i}"] = inputs["Wnb"][i].astype(F16)
        base[f"WbT{i}"] = np.ascontiguousarray(inputs["Wnb"][i].T).astype(F16)
        base[f"W1{i}"] = inputs["Wm1"][i].astype(F16)
        base[f"W2{i}"] = inputs["Wm2"][i].astype(F16)
        wec = np.stack([inputs["Wedge"][i][0], inputs["bedge"][i]])
        base[f"WecA{i}"] = np.ascontiguousarray(wec[0:1].astype(np.float32))
        base[f"WecB{i}"] = np.ascontiguousarray(wec[1:2].astype(np.float32))
        base[f"WecC{i}"] = np.ascontiguousarray(wec.T.astype(np.float32))
        base[f"BnbR{i}"] = np.ascontiguousarray(
            inputs["bnb"][i].astype(F16).reshape(1, F))
        for nm in ("gn", "btn", "ge", "bte", "gnb", "btnb",
                   "gm1", "btm1", "gm2", "btm2"):
            base[f"{nm}{i}"] = col(inputs[nm][i])

    in_maps = []
    for r in range(NCORES):
        m = dict(base)
        m["naT"] = meta["naT"][r]
        m["cip"] = meta["cip"][r]
        m["gidx"] = meta["gidx"][r]
        m["ohc"] = meta["ohc"][r]
        m["ohr"] = meta["ohr"][r]
        m["eav"] = meta["eav_t"][r]
        in_maps.append(m)

    res = bass_utils.run_bass_kernel_spmd(
        nc, in_maps, core_ids=list(range(NCORES)))
    full = np.concatenate([res.results[r]["out"] for r in range(NCORES)], axis=1)
    return np.ascontiguousarray(full.T[:NREAL]).astype(np.float32)


if __name__ == "__main__":
    pass
